# revision 7
# baseline (speedup 1.0000x reference)
"""Trainium2 Bass kernel for nn_Attention_86663850099018.

Math (per batch b, reference semantics):
    xn = x_b / ||x_b rows||                      # (N, E) row-normalized
    S  = xn @ xn.T                               # (N, N) cosine scores, symmetric, |S|<=1
    P  = softmax(S, axis=1)                      # row softmax over keys
    U  = P @ h_b                                 # (N, H)
    out = U / frob_norm(U over all batches)      # the reference's H* factor cancels

Design notes:
  - Rows are relabeled p-major (row = p*16 + t) so DRAM<->SBUF moves are
    contiguous per partition.
  - Both big matmuls run in fp8e4 with perf_mode=DoubleRow (0.5 PE
    cycles/row, contracting 256 rows per instruction):
      * scores: xn pre-scaled by 16 (fp8-friendly range); exp applies
        the 1/256 correction via its scale argument.
      * E @ h: decomposed as (1 + D) @ (h1 + r) where D = exp(S) - 1 is
        small (scores ~ N(0, 1/256)), h1 = fp8(h), r = fp8(h - h1).
        D@h1 + D@r accumulate in PSUM; the exact rank-1 term
        colsum(h1 + r) is added at drain time (computed by 16 one-row
        DoubleRow matmuls against a ones vector).
    fp8 quantization error lands on the small D, not on E -- total rel
    error ~0.4%, well inside the 2e-2 gate.
  - Phase A (scores+exp+D) and phase B are interleaved per column chunk
    with a 3-step lag so the PE stream never waits on ACT/DVE.
  - Softmax denominators come free from the ACT exp's accum_out; the
    sum-of-squares for the global norm from ACT Square accum_out. 1/z and
    the global 1/norm fold into one writeback scale (gz = zinv * ginv).
  - A dummy 4B AllReduce at kernel start warms the CC dispatch path and
    absorbs cross-core launch skew, halving the tail collective cost.
"""

import numpy as np

N, B, E, H = 2048, 8, 256, 512
P = 128
NT = N // P          # 16 row tiles / row blocks
EC = E // P          # 2 contraction chunks
SF = 512             # column-chunk width
NCH = N // SF        # 4 column chunks
TCH = 4              # tiles per input DMA chunk
LAG = 3              # A->B interleave lag (steps)
NCORES = 8

# phase-B group order: ("r", m) = pass over r8 pair m, ("h", m) = h1 pair m.
# h1 groups trail the D-production front (pair m needs D tiles 2m, 2m+1);
# r8 groups lead since r8 is produced independently of phase A.
_ORDER = [
    ("r", 0), ("r", 1), ("h", 0), ("r", 2), ("h", 1), ("r", 3), ("h", 2),
    ("r", 4), ("h", 3), ("r", 5), ("h", 4), ("r", 6), ("h", 5), ("r", 7),
    ("h", 6), ("h", 7),
]

_CACHE = {}


def _build():
    import concourse.mybir as mybir
    import concourse.tile as tile
    from concourse import bacc
    from concourse.masks import make_identity

    f32 = mybir.dt.float32
    f16 = mybir.dt.float16
    f8 = mybir.dt.float8e4
    AF = mybir.ActivationFunctionType
    ALU = mybir.AluOpType
    AX = mybir.AxisListType
    DR = mybir.MatmulPerfMode.DoubleRow

    nc = bacc.Bacc("TRN2", target_bir_lowering=False, debug=False, num_devices=NCORES)

    x_d = nc.dram_tensor("x", [N, E], f32, kind="ExternalInput").ap()
    h_d = nc.dram_tensor("h", [N, H], f32, kind="ExternalInput").ap()
    o_d = nc.dram_tensor("out", [N, H], f32, kind="ExternalOutput").ap()

    # p-major row relabeling: row = p*NT + t
    x_pt = x_d.rearrange("(p t) e -> p t e", t=NT)
    h_pt = h_d.rearrange("(p t) e -> p t e", t=NT)
    o_pt = o_d.rearrange("(p t) e -> p t e", t=NT)

    with tile.TileContext(nc) as tc:
        with (
            tc.tile_pool(name="const", bufs=1) as constp,
            tc.tile_pool(name="big", bufs=1) as bigp,
            tc.tile_pool(name="dramp", bufs=1, space="DRAM") as dramp,
        ):
            x_all = bigp.tile([P, NT, E], f32)
            h32 = bigp.tile([P, NT, H], f32)
            h1 = bigp.tile([P, NT, H], f8)        # fp8(h)
            r8 = bigp.tile([P, NT, H], f8)        # fp8(h - h1)
            xnt = bigp.tile([P, EC, N], f8)       # xn^T * 16, fp8
            d8 = bigp.tile([P, NT, N], f8)        # exp(S) - 1, fp8
            out_sb = bigp.tile([P, NT, H], f32)   # U_raw = E @ h~
            cs_bc = bigp.tile([P, SF], f32)       # colsum(h~) broadcast
            ssqx = bigp.tile([P, NT], f32)
            nrm16 = bigp.tile([P, NT], f32)
            invn16 = bigp.tile([P, NT], f32)
            zpart = bigp.tile([P, NT * NCH], f32)
            zsum = bigp.tile([P, NT], f32)
            zinv = bigp.tile([P, NT], f32)
            zinv2 = bigp.tile([P, NT], f32)
            wss = bigp.tile([P, NT], f32)
            ssqraw = bigp.tile([P, NT], f32)
            ssqcol = bigp.tile([P, 1], f32)
            gz = bigp.tile([P, NT], f32)

            # ---------- input DMAs: 8 transfers over 3 HWDGE queues -------
            # ordered by first-need time of each chunk in the fused loop
            def xs(c):
                return (slice(None), slice(c * TCH, (c + 1) * TCH), slice(None))

            nc.sync.dma_start(x_all[xs(0)], x_pt[xs(0)])
            nc.scalar.dma_start(x_all[xs(1)], x_pt[xs(1)])
            nc.gpsimd.dma_start(h32[xs(0)], h_pt[xs(0)])
            nc.sync.dma_start(x_all[xs(2)], x_pt[xs(2)])
            nc.scalar.dma_start(h32[xs(1)], h_pt[xs(1)])
            nc.gpsimd.dma_start(x_all[xs(3)], x_pt[xs(3)])
            nc.sync.dma_start(h32[xs(2)], h_pt[xs(2)])
            nc.scalar.dma_start(h32[xs(3)], h_pt[xs(3)])

            ident = constp.tile([P, P], f16)
            make_identity(nc, ident[:])
            ones = constp.tile([P, 1], f32)
            nc.vector.memset(ones[:], 1.0)
            ones8 = constp.tile([P, 2, 16], f8)
            nc.vector.memset(ones8[:], 1.0)
            zero1 = constp.tile([1, 1], f32)
            nc.vector.memset(zero1[:], 0.0)

            # ---------- warmup collective (absorbs CC dispatch + skew) ----
            warm_in = dramp.tile([1, 1], f32)
            warm_out = dramp.tile([1, 1], f32, addr_space="Shared")
            nc.gpsimd.dma_start(warm_in[:], zero1[:])
            nc.gpsimd.collective_compute(
                "AllReduce",
                ALU.add,
                replica_groups=[list(range(NCORES))],
                ins=[warm_in.opt()],
                outs=[warm_out.opt()],
            )

            with (
                tc.tile_pool(name="ph0", bufs=3) as ph0,
                tc.tile_pool(name="sqp", bufs=2) as sqp,
                tc.tile_pool(name="escr", bufs=3) as escrp,
                tc.tile_pool(name="psT", bufs=2, space="PSUM") as psT,
                tc.tile_pool(name="psA", bufs=2, space="PSUM") as psAp,
                tc.tile_pool(name="psB", bufs=1, space="PSUM") as psBp,
            ):
                # per-tile normalize + transpose chain (fp8 out)
                def tile_chain(t):
                    sqd = sqp.tile([P, E], f16, tag="sqd")
                    nc.scalar.activation(
                        sqd[:], x_all[:, t, :], AF.Square,
                        accum_out=ssqx[:, t : t + 1],
                    )
                    # nrm16 = sqrt(ssq/256) = ||x||/16 ; invn16 = 16/||x||
                    nc.scalar.activation(
                        nrm16[:, t : t + 1], ssqx[:, t : t + 1], AF.Sqrt,
                        scale=1.0 / 256.0,
                    )
                    nc.vector.reciprocal(
                        invn16[:, t : t + 1], nrm16[:, t : t + 1]
                    )
                    xn = ph0.tile([P, E], f16, tag="xn")
                    nc.vector.tensor_scalar_mul(
                        xn[:], x_all[:, t, :], invn16[:, t : t + 1]
                    )
                    pt = psT.tile([P, EC, P], f16, tag="pt")
                    for c in range(EC):
                        nc.tensor.transpose(
                            pt[:, c, :], xn[:, c * P : (c + 1) * P], ident[:]
                        )
                    nc.vector.tensor_copy(
                        xnt[:, :, t * P : (t + 1) * P], pt[:]
                    )

                for t in range(TCH):
                    tile_chain(t)

                # ---------- fused main loop over column chunks ------------
                for jc in range(NCH):
                    psBs = None
                    for s in range(NT + LAG):
                        if s < NT:
                            i = s
                            ps = psAp.tile([P, SF], f32, tag="psA")
                            nc.tensor.matmul(
                                ps[:],
                                xnt[:, :, i * P : (i + 1) * P],
                                xnt[:, :, jc * SF : (jc + 1) * SF],
                                start=True,
                                stop=True,
                                perf_mode=DR,
                            )
                            ee = escrp.tile([P, SF], f16, tag="ee")
                            nc.scalar.activation(
                                ee[:],
                                ps[:],
                                AF.Exp,
                                scale=1.0 / 256.0,
                                accum_out=zpart[:, i * NCH + jc : i * NCH + jc + 1],
                            )
                            nc.vector.tensor_scalar_add(
                                d8[:, i, jc * SF : (jc + 1) * SF], ee[:], -1.0
                            )
                            if jc == 0:
                                nc.scalar.activation(
                                    h1[:, i, :], h32[:, i, :], AF.Copy
                                )
                                nc.vector.scalar_tensor_tensor(
                                    r8[:, i, :], h32[:, i, :], 1.0,
                                    h1[:, i, :], ALU.mult, ALU.subtract,
                                )
                        if s >= LAG:
                            q = s - LAG
                            kind, m = _ORDER[q]
                            src = h1 if kind == "h" else r8
                            if q == 0:
                                psBs = [
                                    psBp.tile(
                                        [P, H], f32, name=f"psB{j}", tag=f"psB{j}"
                                    )
                                    for j in range(NCH)
                                ]
                            for j in range(NCH):
                                jj = jc * NCH + j
                                nc.tensor.matmul(
                                    psBs[j][:],
                                    d8[:, 2 * m : 2 * m + 2, jj * P : (jj + 1) * P],
                                    src[:, 2 * m : 2 * m + 2, :],
                                    start=(q == 0),
                                    stop=(q == len(_ORDER) - 1),
                                    perf_mode=DR,
                                )
                        if jc == 0 and s + TCH < NT:
                            tile_chain(s + TCH)

                    if jc == 0:
                        # exact rank-1 colsum(h1 + r8) via one-row DR matmuls
                        psC = psAp.tile([P, SF], f32, name="psC", tag="psA")
                        for m in range(8):
                            nc.tensor.matmul(
                                psC[:1, :], ones8[:, :, 0:1],
                                h1[:, 2 * m : 2 * m + 2, :],
                                start=(m == 0), stop=False, perf_mode=DR,
                            )
                        for m in range(8):
                            nc.tensor.matmul(
                                psC[:1, :], ones8[:, :, 0:1],
                                r8[:, 2 * m : 2 * m + 2, :],
                                start=False, stop=(m == 7), perf_mode=DR,
                            )
                        cs1 = constp.tile([1, SF], f32)
                        nc.scalar.copy(cs1[:], psC[:1, :])
                        nc.gpsimd.partition_broadcast(cs_bc[:], cs1[:])

                    for j in range(NCH):
                        jj = jc * NCH + j
                        nc.vector.tensor_add(
                            out_sb[:, jj, :], psBs[j][:], cs_bc[:]
                        )
                        sq2 = sqp.tile([P, H], f16, tag="sq2")
                        nc.scalar.activation(
                            sq2[:], out_sb[:, jj, :], AF.Square,
                            accum_out=ssqraw[:, jj : jj + 1],
                        )

            # ---------------- tail: global norm + writeback ---------------
            with (
                tc.tile_pool(name="tailp", bufs=2) as tailp,
                tc.tile_pool(name="psS", bufs=1, space="PSUM") as psS,
            ):
                nc.vector.tensor_reduce(
                    zsum[:],
                    zpart[:].rearrange("p (i j) -> p i j", j=NCH),
                    axis=AX.X,
                    op=ALU.add,
                )
                nc.vector.reciprocal(zinv[:], zsum[:])
                nc.vector.tensor_mul(zinv2[:], zinv[:], zinv[:])
                nc.vector.tensor_mul(wss[:], zinv2[:], ssqraw[:])
                nc.vector.tensor_reduce(
                    ssqcol[:], wss[:], axis=AX.X, op=ALU.add
                )
                ps1 = psS.tile([1, 1], f32, tag="ps1")
                nc.tensor.matmul(ps1[:], ones[:], ssqcol[:], start=True, stop=True)
                ss11 = tailp.tile([1, 1], f32, tag="ss11")
                nc.scalar.copy(ss11[:], ps1[:])

                cc_in = dramp.tile([1, 1], f32)
                cc_out = dramp.tile([1, 1], f32, addr_space="Shared")
                nc.gpsimd.dma_start(cc_in[:], ss11[:])
                nc.gpsimd.collective_compute(
                    "AllReduce",
                    ALU.add,
                    replica_groups=[list(range(NCORES))],
                    ins=[cc_in.opt()],
                    outs=[cc_out.opt()],
                )
                agg = tailp.tile([1, 1], f32, tag="agg")
                nc.sync.dma_start(agg[:], cc_out[:])

                glen = tailp.tile([1, 1], f32, tag="glen")
                ginv = tailp.tile([1, 1], f32, tag="ginv")
                nc.scalar.activation(glen[:], agg[:], AF.Sqrt)
                nc.vector.reciprocal(ginv[:], glen[:])
                gbc = tailp.tile([P, 1], f32, tag="gbc")
                nc.gpsimd.partition_broadcast(gbc[:], ginv[:])
                nc.vector.tensor_scalar_mul(gz[:], zinv[:], gbc[:])

                qengs = [nc.sync, nc.scalar, nc.gpsimd]
                for jj in range(NT):
                    blk = out_sb[:, jj, :]
                    if jj % 2 == 0:
                        nc.vector.tensor_scalar_mul(
                            blk, blk, gz[:, jj : jj + 1]
                        )
                    else:
                        nc.scalar.activation(
                            blk, blk, AF.Copy, scale=gz[:, jj : jj + 1]
                        )
                    qengs[jj % 3].dma_start(o_pt[:, jj, :], blk)

    nc.compile()
    return nc


def _get_nc():
    if "nc" not in _CACHE:
        _CACHE["nc"] = _build()
    return _CACHE["nc"]


def _in_maps(x, h):
    return [
        {
            "x": np.ascontiguousarray(x[:, c, :]),
            "h": np.ascontiguousarray(h[:, c, :]),
        }
        for c in range(NCORES)
    ]


def kernel(x, h):
    from concourse.bass_utils import run_bass_kernel_spmd

    x = np.asarray(x, dtype=np.float32)
    h = np.asarray(h, dtype=np.float32)
    assert x.shape == (N, B, E) and h.shape == (N, B, H)

    nc = _get_nc()
    res = run_bass_kernel_spmd(nc, _in_maps(x, h), core_ids=list(range(NCORES)))
    out = np.empty((N, B, H), dtype=np.float32)
    for c in range(NCORES):
        out[:, c, :] = res.results[c]["out"]
    return out


# Exposed for test.py: run once with tracing to get hardware exec time.
def run_traced(x, h):
    import os
    import shutil

    from concourse.bass_utils import run_bass_kernel_spmd

    x = np.asarray(x, dtype=np.float32)
    h = np.asarray(h, dtype=np.float32)
    nc = _get_nc()
    tdir = "/root/problem/trace_out"
    shutil.rmtree(tdir, ignore_errors=True)
    os.makedirs(tdir, exist_ok=True)
    res = run_bass_kernel_spmd(
        nc, _in_maps(x, h), core_ids=list(range(NCORES)), trace=True, tmpdir=tdir
    )
    out = np.empty((N, B, H), dtype=np.float32)
    for c in range(NCORES):
        out[:, c, :] = res.results[c]["out"]
    return out, res


# revision 9
# speedup vs baseline: 1.1759x; 1.1759x over previous
"""Trainium2 Bass kernel for nn_Attention_86663850099018.

Math (per batch b, reference semantics):
    xn = x_b / ||x_b rows||                      # (N, E) row-normalized
    S  = xn @ xn.T                               # (N, N) cosine scores, symmetric, |S|<=1
    P  = softmax(S, axis=1)                      # row softmax over keys
    U  = P @ h_b                                 # (N, H)
    out = U / frob_norm(U over all batches)      # the reference's H* factor cancels

Design notes:
  - Rows are relabeled p-major (row = p*16 + t) so DRAM<->SBUF moves are
    contiguous per partition.
  - Both big matmuls run in fp8e4 with perf_mode=DoubleRow, whose value is
    2x contraction per instruction (K=256/instr):
      * scores: one DR matmul per (row-block, col-chunk) contracts all of
        E=256. xn is pre-scaled by 16 (fp8-friendly range); exp applies
        the 1/256 correction via its scale argument.
      * E @ h: decomposed as U = colsum(h) + D @ h1 where D = exp(S) - 1
        is small (scores ~ N(0, 1/256)) and h1 = fp8(h). Quantization
        error of BOTH D and h enters only through the tiny D product
        (~0.2% each); the rank-1 colsum(h) term is computed exactly from
        the f32 h. DR pairs two row-tiles per instruction: 8 matmuls per
        output block instead of 16.
  - ACT table discipline: the only table-anchored function used is Exp
    (+Ln); Copy/Square are fillers present in every set. 1/sqrt(a) is
    computed as exp(-0.5*ln(a)) to stay inside natural_log_exp_and_others
    and avoid ~2.7us table switches per Sqrt.
  - Phase A (scores+exp+D) and phase B interleave per column chunk with a
    lag so the PE stream never waits on ACT/DVE; row sums (softmax denom)
    come free from exp's accum_out, sum-of-squares from Square accum_out.
    1/z and the global 1/norm fold into one writeback scale.
  - A dummy 4B AllReduce at kernel start warms the CC dispatch path and
    absorbs cross-core launch skew, halving the tail collective cost.
"""

import numpy as np

N, B, E, H = 2048, 8, 256, 512
P = 128
NT = N // P          # 16 row tiles / row blocks
EC = E // P          # 2 contraction chunks
SF = 512             # column-chunk width
NCH = N // SF        # 4 column chunks
TCH = 4              # tiles per input DMA chunk
NCORES = 8

_CACHE = {}


def _build():
    import concourse.mybir as mybir
    import concourse.tile as tile
    from concourse import bacc
    from concourse.masks import make_identity

    f32 = mybir.dt.float32
    f16 = mybir.dt.float16
    f8 = mybir.dt.float8e4
    AF = mybir.ActivationFunctionType
    ALU = mybir.AluOpType
    AX = mybir.AxisListType
    DR = mybir.MatmulPerfMode.DoubleRow

    nc = bacc.Bacc("TRN2", target_bir_lowering=False, debug=False, num_devices=NCORES)

    x_d = nc.dram_tensor("x", [N, E], f32, kind="ExternalInput").ap()
    h_d = nc.dram_tensor("h", [N, H], f32, kind="ExternalInput").ap()
    o_d = nc.dram_tensor("out", [N, H], f32, kind="ExternalOutput").ap()

    # p-major row relabeling: row = p*NT + t
    x_pt = x_d.rearrange("(p t) e -> p t e", t=NT)
    h_pt = h_d.rearrange("(p t) e -> p t e", t=NT)
    o_pt = o_d.rearrange("(p t) e -> p t e", t=NT)

    with tile.TileContext(nc) as tc:
        with (
            tc.tile_pool(name="const", bufs=1) as constp,
            tc.tile_pool(name="big", bufs=1) as bigp,
            tc.tile_pool(name="dramp", bufs=1, space="DRAM") as dramp,
        ):
            x_all = bigp.tile([P, NT, E], f32)
            h32 = bigp.tile([P, NT, H], f32)
            h1 = bigp.tile([P, NT, H], f8)        # fp8(h)
            xnt = bigp.tile([P, EC, N], f8)       # xn^T * 16, fp8
            d8 = bigp.tile([P, NT, N], f8)        # exp(S) - 1, fp8
            out_sb = bigp.tile([P, NT, H], f32)   # U_raw
            acc = bigp.tile([P, H], f32)          # running colsum of h
            acc16 = bigp.tile([P, H], f16)
            cs_bc = bigp.tile([P, SF], f32)       # colsum(h) broadcast
            ssqx = bigp.tile([P, NT], f32)
            lnv = bigp.tile([P, NT], f32)
            invn16 = bigp.tile([P, NT], f32)
            zpart = bigp.tile([P, NT * NCH], f32)
            zsum = bigp.tile([P, NT], f32)
            zinv = bigp.tile([P, NT], f32)
            zinv2 = bigp.tile([P, NT], f32)
            wss = bigp.tile([P, NT], f32)
            ssqraw = bigp.tile([P, NT], f32)
            ssqcol = bigp.tile([P, 1], f32)
            gz = bigp.tile([P, NT], f32)

            # ---------- input DMAs: 8 transfers over 3 HWDGE queues -------
            # ordered by first-need time of each chunk in the fused loop
            def xs(c):
                return (slice(None), slice(c * TCH, (c + 1) * TCH), slice(None))

            nc.sync.dma_start(x_all[xs(0)], x_pt[xs(0)])
            nc.scalar.dma_start(x_all[xs(1)], x_pt[xs(1)])
            nc.gpsimd.dma_start(h32[xs(0)], h_pt[xs(0)])
            nc.sync.dma_start(x_all[xs(2)], x_pt[xs(2)])
            nc.scalar.dma_start(h32[xs(1)], h_pt[xs(1)])
            nc.gpsimd.dma_start(x_all[xs(3)], x_pt[xs(3)])
            nc.sync.dma_start(h32[xs(2)], h_pt[xs(2)])
            nc.scalar.dma_start(h32[xs(3)], h_pt[xs(3)])

            ident = constp.tile([P, P], f16)
            make_identity(nc, ident[:])
            ones = constp.tile([P, 1], f32)
            nc.vector.memset(ones[:], 1.0)
            ones16 = constp.tile([P, 1], f16)
            nc.vector.memset(ones16[:], 1.0)
            zero1 = constp.tile([1, 1], f32)
            nc.vector.memset(zero1[:], 0.0)

            # ---------- warmup collective (absorbs CC dispatch + skew) ----
            warm_in = dramp.tile([1, 1], f32)
            warm_out = dramp.tile([1, 1], f32, addr_space="Shared")
            nc.gpsimd.dma_start(warm_in[:], zero1[:])
            nc.gpsimd.collective_compute(
                "AllReduce",
                ALU.add,
                replica_groups=[list(range(NCORES))],
                ins=[warm_in.opt()],
                outs=[warm_out.opt()],
            )

            with (
                tc.tile_pool(name="ph0", bufs=3) as ph0,
                tc.tile_pool(name="sqp", bufs=2) as sqp,
                tc.tile_pool(name="escr", bufs=3) as escrp,
                tc.tile_pool(name="psT", bufs=2, space="PSUM") as psT,
                tc.tile_pool(name="psA", bufs=2, space="PSUM") as psAp,
                tc.tile_pool(name="psB", bufs=1, space="PSUM") as psBp,
            ):
                # normalize + transpose for one 4-tile chunk (fp8 out).
                # invn16 = 16/||x|| = exp(-0.5 * ln(ssq/256)) -- keeps ACT
                # inside the natural_log_exp table set (no Sqrt loads).
                def chunk_chain(c):
                    t0 = c * TCH
                    for t in range(t0, t0 + TCH):
                        sqd = sqp.tile([P, E], f16, tag="sqd")
                        nc.scalar.activation(
                            sqd[:], x_all[:, t, :], AF.Square,
                            accum_out=ssqx[:, t : t + 1],
                        )
                    nc.scalar.activation(
                        lnv[:, t0 : t0 + TCH], ssqx[:, t0 : t0 + TCH],
                        AF.Ln, scale=1.0 / 256.0,
                    )
                    nc.scalar.activation(
                        invn16[:, t0 : t0 + TCH], lnv[:, t0 : t0 + TCH],
                        AF.Exp, scale=-0.5,
                    )
                    for t in range(t0, t0 + TCH):
                        xn = ph0.tile([P, E], f16, tag="xn")
                        nc.vector.tensor_scalar_mul(
                            xn[:], x_all[:, t, :], invn16[:, t : t + 1]
                        )
                        pt = psT.tile([P, EC, P], f16, tag="pt")
                        for cc in range(EC):
                            nc.tensor.transpose(
                                pt[:, cc, :], xn[:, cc * P : (cc + 1) * P],
                                ident[:],
                            )
                        nc.vector.tensor_copy(
                            xnt[:, :, t * P : (t + 1) * P], pt[:]
                        )

                chunk_chain(0)

                # ---------- fused main loop over column chunks ------------
                for jc in range(NCH):
                    psBs = None
                    for s in range(NT + 3):
                        if s < NT:
                            i = s
                            ps = psAp.tile([P, SF], f32, tag="psA")
                            nc.tensor.matmul(
                                ps[:],
                                xnt[:, :, i * P : (i + 1) * P],
                                xnt[:, :, jc * SF : (jc + 1) * SF],
                                start=True,
                                stop=True,
                                perf_mode=DR,
                            )
                            ee = escrp.tile([P, SF], f16, tag="ee")
                            nc.scalar.activation(
                                ee[:],
                                ps[:],
                                AF.Exp,
                                scale=1.0 / 256.0,
                                accum_out=zpart[:, i * NCH + jc : i * NCH + jc + 1],
                            )
                            nc.vector.tensor_scalar_add(
                                d8[:, i, jc * SF : (jc + 1) * SF], ee[:], -1.0
                            )
                            if jc == 0:
                                nc.scalar.activation(
                                    h1[:, i, :], h32[:, i, :], AF.Copy
                                )
                                if i == 1:
                                    nc.vector.tensor_add(
                                        acc[:], h32[:, 0, :], h32[:, 1, :]
                                    )
                                elif i > 1:
                                    nc.vector.tensor_add(
                                        acc[:], acc[:], h32[:, i, :]
                                    )
                        if s >= 3 and (s - 3) % 2 == 0:
                            m = (s - 3) // 2
                            if m == 0:
                                psBs = [
                                    psBp.tile(
                                        [P, H], f32, name=f"psB{j}", tag=f"psB{j}"
                                    )
                                    for j in range(NCH)
                                ]
                            for j in range(NCH):
                                jj = jc * NCH + j
                                nc.tensor.matmul(
                                    psBs[j][:],
                                    d8[:, 2 * m : 2 * m + 2, jj * P : (jj + 1) * P],
                                    h1[:, 2 * m : 2 * m + 2, :],
                                    start=(m == 0),
                                    stop=(m == 7),
                                    perf_mode=DR,
                                )
                        if jc == 0 and s in (0, 4, 8):
                            chunk_chain(s // 4 + 1)

                    if jc == 0:
                        # exact rank-1 colsum(h): partition-reduce the f32
                        # tile-tree sum via a single ones matmul
                        nc.vector.tensor_copy(acc16[:], acc[:])
                        psC = psAp.tile([P, SF], f32, name="psC", tag="psA")
                        nc.tensor.matmul(
                            psC[:1, :], ones16[:], acc16[:],
                            start=True, stop=True,
                        )
                        cs1 = constp.tile([1, SF], f32)
                        nc.scalar.copy(cs1[:], psC[:1, :])
                        nc.gpsimd.partition_broadcast(cs_bc[:], cs1[:])

                    for j in range(NCH):
                        jj = jc * NCH + j
                        nc.vector.tensor_add(
                            out_sb[:, jj, :], psBs[j][:], cs_bc[:]
                        )
                        sq2 = sqp.tile([P, H], f16, tag="sq2")
                        nc.scalar.activation(
                            sq2[:], out_sb[:, jj, :], AF.Square,
                            accum_out=ssqraw[:, jj : jj + 1],
                        )

            # ---------------- tail: global norm + writeback ---------------
            with (
                tc.tile_pool(name="tailp", bufs=2) as tailp,
                tc.tile_pool(name="psS", bufs=1, space="PSUM") as psS,
            ):
                nc.vector.tensor_reduce(
                    zsum[:],
                    zpart[:].rearrange("p (i j) -> p i j", j=NCH),
                    axis=AX.X,
                    op=ALU.add,
                )
                nc.vector.reciprocal(zinv[:], zsum[:])
                nc.vector.tensor_mul(zinv2[:], zinv[:], zinv[:])
                nc.vector.tensor_mul(wss[:], zinv2[:], ssqraw[:])
                nc.vector.tensor_reduce(
                    ssqcol[:], wss[:], axis=AX.X, op=ALU.add
                )
                ps1 = psS.tile([1, 1], f32, tag="ps1")
                nc.tensor.matmul(ps1[:], ones[:], ssqcol[:], start=True, stop=True)
                ss11 = tailp.tile([1, 1], f32, tag="ss11")
                nc.scalar.copy(ss11[:], ps1[:])

                cc_in = dramp.tile([1, 1], f32)
                cc_out = dramp.tile([1, 1], f32, addr_space="Shared")
                nc.gpsimd.dma_start(cc_in[:], ss11[:])
                nc.gpsimd.collective_compute(
                    "AllReduce",
                    ALU.add,
                    replica_groups=[list(range(NCORES))],
                    ins=[cc_in.opt()],
                    outs=[cc_out.opt()],
                )
                agg = tailp.tile([1, 1], f32, tag="agg")
                nc.sync.dma_start(agg[:], cc_out[:])

                # ginv = 1/sqrt(agg) = exp(-0.5 ln(agg)); stays in-set
                lnag = tailp.tile([1, 1], f32, tag="lnag")
                ginv = tailp.tile([1, 1], f32, tag="ginv")
                nc.scalar.activation(lnag[:], agg[:], AF.Ln)
                nc.scalar.activation(ginv[:], lnag[:], AF.Exp, scale=-0.5)
                gbc = tailp.tile([P, 1], f32, tag="gbc")
                nc.gpsimd.partition_broadcast(gbc[:], ginv[:])
                nc.vector.tensor_scalar_mul(gz[:], zinv[:], gbc[:])

                qengs = [nc.sync, nc.scalar, nc.gpsimd]
                for jj in range(NT):
                    blk = out_sb[:, jj, :]
                    if jj % 2 == 0:
                        nc.vector.tensor_scalar_mul(
                            blk, blk, gz[:, jj : jj + 1]
                        )
                    else:
                        nc.scalar.activation(
                            blk, blk, AF.Copy, scale=gz[:, jj : jj + 1]
                        )
                    qengs[jj % 3].dma_start(o_pt[:, jj, :], blk)

    nc.compile()
    return nc


def _get_nc():
    if "nc" not in _CACHE:
        _CACHE["nc"] = _build()
    return _CACHE["nc"]


def _in_maps(x, h):
    return [
        {
            "x": np.ascontiguousarray(x[:, c, :]),
            "h": np.ascontiguousarray(h[:, c, :]),
        }
        for c in range(NCORES)
    ]


def kernel(x, h):
    from concourse.bass_utils import run_bass_kernel_spmd

    x = np.asarray(x, dtype=np.float32)
    h = np.asarray(h, dtype=np.float32)
    assert x.shape == (N, B, E) and h.shape == (N, B, H)

    nc = _get_nc()
    res = run_bass_kernel_spmd(nc, _in_maps(x, h), core_ids=list(range(NCORES)))
    out = np.empty((N, B, H), dtype=np.float32)
    for c in range(NCORES):
        out[:, c, :] = res.results[c]["out"]
    return out


# Exposed for test.py: run once with tracing to get hardware exec time.
def run_traced(x, h):
    import os
    import shutil

    from concourse.bass_utils import run_bass_kernel_spmd

    x = np.asarray(x, dtype=np.float32)
    h = np.asarray(h, dtype=np.float32)
    nc = _get_nc()
    tdir = "/root/problem/trace_out"
    shutil.rmtree(tdir, ignore_errors=True)
    os.makedirs(tdir, exist_ok=True)
    res = run_bass_kernel_spmd(
        nc, _in_maps(x, h), core_ids=list(range(NCORES)), trace=True, tmpdir=tdir
    )
    out = np.empty((N, B, H), dtype=np.float32)
    for c in range(NCORES):
        out[:, c, :] = res.results[c]["out"]
    return out, res


# revision 16
# speedup vs baseline: 1.2156x; 1.0338x over previous
"""Trainium2 Bass kernel for nn_Attention_86663850099018.

Math (per batch b, reference semantics):
    xn = x_b / ||x_b rows||                      # (N, E) row-normalized
    S  = xn @ xn.T                               # (N, N) cosine scores, symmetric, |S|<=1
    P  = softmax(S, axis=1)                      # row softmax over keys
    U  = P @ h_b                                 # (N, H)
    out = U / frob_norm(U over all batches)      # the reference's H* factor cancels

Design notes:
  - Rows are relabeled p-major (row = p*16 + t) so DRAM<->SBUF moves are
    contiguous per partition.
  - Both big matmuls run in fp8e4 with perf_mode=DoubleRow, whose value is
    2x contraction per instruction (K=256/instr):
      * scores: one DR matmul per (row-block, col-chunk) contracts all of
        E=256. xn is pre-scaled by 16 (fp8-friendly range); exp applies
        the 1/256 correction via its scale argument.
      * E @ h: decomposed as U = colsum(h) + D @ h1 where D = exp(S) - 1
        is small (scores ~ N(0, 1/256)) and h1 = fp8(h). Quantization
        error of BOTH D and h enters only through the tiny D product
        (~0.2% each); the rank-1 colsum(h) term is computed exactly from
        the f32 h. DR pairs two row-tiles per instruction: 8 matmuls per
        output block instead of 16.
  - ACT table discipline: the only table-anchored function used is Exp
    (+Ln); Copy/Square are fillers present in every set. 1/sqrt(a) is
    computed as exp(-0.5*ln(a)) to stay inside natural_log_exp_and_others
    and avoid ~2.7us table switches per Sqrt.
  - Phase A (scores+exp+D) and phase B interleave per column chunk with a
    lag so the PE stream never waits on ACT/DVE; row sums (softmax denom)
    come free from exp's accum_out, sum-of-squares from Square accum_out.
    1/z and the global 1/norm fold into one writeback scale.
  - A dummy 4B AllReduce at kernel start warms the CC dispatch path and
    absorbs cross-core launch skew, halving the tail collective cost.
"""

import numpy as np

N, B, E, H = 2048, 8, 256, 512
P = 128
NT = N // P          # 16 row tiles / row blocks
EC = E // P          # 2 contraction chunks
SF = 512             # column-chunk width
NCH = N // SF        # 4 column chunks
TCH = 4              # tiles per input DMA chunk
NCORES = 8

_CACHE = {}


def _build():
    import concourse.mybir as mybir
    import concourse.tile as tile
    from concourse import bacc
    from concourse.masks import make_identity

    f32 = mybir.dt.float32
    f16 = mybir.dt.float16
    f8 = mybir.dt.float8e4
    AF = mybir.ActivationFunctionType
    ALU = mybir.AluOpType
    AX = mybir.AxisListType
    DR = mybir.MatmulPerfMode.DoubleRow

    nc = bacc.Bacc("TRN2", target_bir_lowering=False, debug=False, num_devices=NCORES)

    x_d = nc.dram_tensor("x", [N, E], f32, kind="ExternalInput").ap()
    h_d = nc.dram_tensor("h", [N, H], f32, kind="ExternalInput").ap()
    o_d = nc.dram_tensor("out", [N, H], f32, kind="ExternalOutput").ap()

    # p-major row relabeling: row = p*NT + t
    x_pt = x_d.rearrange("(p t) e -> p t e", t=NT)
    h_pt = h_d.rearrange("(p t) e -> p t e", t=NT)
    o_pt = o_d.rearrange("(p t) e -> p t e", t=NT)

    with tile.TileContext(nc) as tc:
        with (
            tc.tile_pool(name="const", bufs=1) as constp,
            tc.tile_pool(name="big", bufs=1) as bigp,
            tc.tile_pool(name="dramp", bufs=1, space="DRAM") as dramp,
        ):
            x_all = bigp.tile([P, NT, E], f32)
            h32 = bigp.tile([P, NT, H], f32)
            h1 = bigp.tile([P, NT, H], f8)        # fp8(h)
            xnt = bigp.tile([P, EC, N], f8)       # xn^T * 16, fp8
            d8 = bigp.tile([P, NT, N], f8)        # exp(S) - 1, fp8
            out_sb = bigp.tile([P, NT, H], f32)   # U_raw
            acc = bigp.tile([P, H], f32)          # running colsum of h
            acc16 = bigp.tile([P, H], f16)
            cs_bc = bigp.tile([P, SF], f32)       # colsum(h) broadcast
            ssqx = bigp.tile([P, NT], f32)
            lnv = bigp.tile([P, NT], f32)
            invn16 = bigp.tile([P, NT], f32)
            zpart = bigp.tile([P, NT * NCH], f32)
            zsum = bigp.tile([P, NT], f32)
            zinv = bigp.tile([P, NT], f32)
            zinv2 = bigp.tile([P, NT], f32)
            wss = bigp.tile([P, NT], f32)
            ssqraw = bigp.tile([P, NT], f32)
            ssqcol = bigp.tile([P, 1], f32)
            gz = bigp.tile([P, NT], f32)

            # ---------- input DMAs: 8 transfers over 3 HWDGE queues -------
            # ordered by first-need time of each chunk in the fused loop
            def xs(c):
                return (slice(None), slice(c * TCH, (c + 1) * TCH), slice(None))

            nc.sync.dma_start(x_all[xs(0)], x_pt[xs(0)])
            nc.scalar.dma_start(x_all[xs(1)], x_pt[xs(1)])
            nc.gpsimd.dma_start(x_all[xs(2)], x_pt[xs(2)])
            nc.gpsimd.dma_start(x_all[xs(3)], x_pt[xs(3)])
            nc.scalar.dma_start(h32[xs(0)], h_pt[xs(0)])
            nc.sync.dma_start(h32[xs(1)], h_pt[xs(1)])
            nc.scalar.dma_start(h32[xs(2)], h_pt[xs(2)])
            nc.sync.dma_start(h32[xs(3)], h_pt[xs(3)])

            ident = constp.tile([P, P], f16)
            make_identity(nc, ident[:])
            ones = constp.tile([P, 1], f32)
            nc.vector.memset(ones[:], 1.0)
            ones16 = constp.tile([P, 1], f16)
            nc.vector.memset(ones16[:], 1.0)
            zero1 = constp.tile([1, 1], f32)
            nc.vector.memset(zero1[:], 0.0)
            # preload both ACT table slots (exp -> sel0, ln -> sel1) while
            # the input DMAs are in flight, so the real Ln/Exp pair later
            # pays at most one reload
            dscr = constp.tile([1, 1], f32)
            nc.scalar.activation(dscr[:], ones[:1, :1], AF.Exp)
            nc.scalar.activation(dscr[:], ones[:1, :1], AF.Ln)

            # ---------- warmup collective (absorbs CC dispatch + skew) ----
            warm_in = dramp.tile([1, 1], f32)
            warm_out = dramp.tile([1, 1], f32, addr_space="Shared")
            nc.gpsimd.dma_start(warm_in[:], zero1[:])
            nc.gpsimd.collective_compute(
                "AllReduce",
                ALU.add,
                replica_groups=[list(range(NCORES))],
                ins=[warm_in.opt()],
                outs=[warm_out.opt()],
            )

            with (
                tc.tile_pool(name="ph0", bufs=3) as ph0,
                tc.tile_pool(name="sqp", bufs=2) as sqp,
                tc.tile_pool(name="escr", bufs=3) as escrp,
                tc.tile_pool(name="psT", bufs=2, space="PSUM") as psT,
                tc.tile_pool(name="psA", bufs=2, space="PSUM") as psAp,
                tc.tile_pool(name="psB", bufs=1, space="PSUM") as psBp,
            ):
                # phase 0: per-tile sum-of-squares on DVE as x chunks land,
                # then ONE batched invn16 = 16/||x|| = exp(-0.5*ln(ssq/256))
                # pair on ACT (Sqrt would thrash the activation tables).
                for t in range(NT):
                    sqd = sqp.tile([P, E], f16, tag="sqd")
                    nc.vector.scalar_tensor_tensor(
                        sqd[:], x_all[:, t, :], 1.0, x_all[:, t, :],
                        ALU.mult, ALU.mult,
                        accum_out=ssqx[:, t : t + 1],
                    )
                nc.scalar.activation(lnv[:], ssqx[:], AF.Ln, scale=1.0 / 256.0)
                nc.scalar.activation(invn16[:], lnv[:], AF.Exp, scale=-0.5)

                # normalize + transpose one tile into fp8 xn^T
                def tile_finish(t):
                    xn = ph0.tile([P, E], f16, tag="xn")
                    nc.vector.tensor_scalar_mul(
                        xn[:], x_all[:, t, :], invn16[:, t : t + 1]
                    )
                    pt = psT.tile([P, EC, P], f16, tag="pt")
                    for cc in range(EC):
                        nc.tensor.transpose(
                            pt[:, cc, :], xn[:, cc * P : (cc + 1) * P],
                            ident[:],
                        )
                    nc.vector.tensor_copy(
                        xnt[:, :, t * P : (t + 1) * P], pt[:]
                    )

                for t in range(TCH):
                    tile_finish(t)

                # ---------- fused main loop over column chunks ------------
                for jc in range(NCH):
                    psBs = None
                    for s in range(NT + 3):
                        if s < NT:
                            i = s
                            ps = psAp.tile([P, SF], f32, tag="psA")
                            nc.tensor.matmul(
                                ps[:],
                                xnt[:, :, i * P : (i + 1) * P],
                                xnt[:, :, jc * SF : (jc + 1) * SF],
                                start=True,
                                stop=True,
                                perf_mode=DR,
                            )
                            if jc == 0:
                                nc.scalar.activation(
                                    h1[:, i, :], h32[:, i, :], AF.Copy
                                )
                            ee = escrp.tile([P, SF], f16, tag="ee")
                            nc.scalar.activation(
                                ee[:], ps[:], AF.Exp, scale=1.0 / 256.0
                            )
                            # d8 = E - 1; accum gives z_chunk - SF for free
                            nc.vector.tensor_scalar(
                                d8[:, i, jc * SF : (jc + 1) * SF],
                                ee[:],
                                -1.0,
                                1.0,
                                ALU.add,
                                ALU.mult,
                                accum_out=zpart[:, i * NCH + jc : i * NCH + jc + 1],
                            )
                            if jc == 0:
                                if i == 1:
                                    nc.vector.tensor_add(
                                        acc[:], h32[:, 0, :], h32[:, 1, :]
                                    )
                                elif i > 1:
                                    nc.vector.tensor_add(
                                        acc[:], acc[:], h32[:, i, :]
                                    )
                        if s >= 3 and (s - 3) % 2 == 0:
                            m = (s - 3) // 2
                            if m == 0:
                                psBs = [
                                    psBp.tile(
                                        [P, H], f32, name=f"psB{j}", tag=f"psB{j}"
                                    )
                                    for j in range(NCH)
                                ]
                            for j in range(NCH):
                                jj = jc * NCH + j
                                nc.tensor.matmul(
                                    psBs[j][:],
                                    d8[:, 2 * m : 2 * m + 2, jj * P : (jj + 1) * P],
                                    h1[:, 2 * m : 2 * m + 2, :],
                                    start=(m == 0),
                                    stop=(m == 7),
                                    perf_mode=DR,
                                )
                        if jc == 0 and s + TCH < NT:
                            tile_finish(s + TCH)

                    if jc == 0:
                        # exact rank-1 colsum(h): partition-reduce the f32
                        # tile-tree sum via a single ones matmul
                        nc.vector.tensor_copy(acc16[:], acc[:])
                        psC = psAp.tile([P, SF], f32, name="psC", tag="psA")
                        nc.tensor.matmul(
                            psC[:1, :], ones16[:], acc16[:],
                            start=True, stop=True,
                        )
                        cs1 = constp.tile([1, SF], f32)
                        nc.scalar.copy(cs1[:], psC[:1, :])
                        nc.gpsimd.partition_broadcast(cs_bc[:], cs1[:])

                    for j in range(NCH):
                        jj = jc * NCH + j
                        nc.vector.tensor_add(
                            out_sb[:, jj, :], psBs[j][:], cs_bc[:]
                        )
                        sq2 = sqp.tile([P, H], f16, tag="sq2")
                        nc.scalar.activation(
                            sq2[:], out_sb[:, jj, :], AF.Square,
                            accum_out=ssqraw[:, jj : jj + 1],
                        )

            # ---------------- tail: global norm + writeback ---------------
            with (
                tc.tile_pool(name="tailp", bufs=2) as tailp,
                tc.tile_pool(name="psS", bufs=1, space="PSUM") as psS,
            ):
                nc.vector.tensor_reduce(
                    zsum[:],
                    zpart[:].rearrange("p (i j) -> p i j", j=NCH),
                    axis=AX.X,
                    op=ALU.add,
                )
                # zpart accumulated E-1, so add back the N ones per row
                nc.vector.tensor_scalar_add(zsum[:], zsum[:], float(N))
                nc.vector.reciprocal(zinv[:], zsum[:])
                nc.vector.tensor_mul(zinv2[:], zinv[:], zinv[:])
                nc.vector.tensor_mul(wss[:], zinv2[:], ssqraw[:])
                nc.vector.tensor_reduce(
                    ssqcol[:], wss[:], axis=AX.X, op=ALU.add
                )
                ps1 = psS.tile([1, 1], f32, tag="ps1")
                nc.tensor.matmul(ps1[:], ones[:], ssqcol[:], start=True, stop=True)
                ss11 = tailp.tile([1, 1], f32, tag="ss11")
                nc.scalar.copy(ss11[:], ps1[:])

                cc_in = dramp.tile([1, 1], f32)
                cc_out = dramp.tile([1, 1], f32, addr_space="Shared")
                nc.gpsimd.dma_start(cc_in[:], ss11[:])
                nc.gpsimd.collective_compute(
                    "AllReduce",
                    ALU.add,
                    replica_groups=[list(range(NCORES))],
                    ins=[cc_in.opt()],
                    outs=[cc_out.opt()],
                )
                agg = tailp.tile([1, 1], f32, tag="agg")
                nc.sync.dma_start(agg[:], cc_out[:])

                # ginv = 1/sqrt(agg) = exp(-0.5 ln(agg)); stays in-set
                lnag = tailp.tile([1, 1], f32, tag="lnag")
                ginv = tailp.tile([1, 1], f32, tag="ginv")
                nc.scalar.activation(lnag[:], agg[:], AF.Ln)
                nc.scalar.activation(ginv[:], lnag[:], AF.Exp, scale=-0.5)
                gbc = tailp.tile([P, 1], f32, tag="gbc")
                nc.gpsimd.partition_broadcast(gbc[:], ginv[:])
                nc.vector.tensor_scalar_mul(gz[:], zinv[:], gbc[:])

                qengs = [nc.sync, nc.scalar, nc.gpsimd]
                for jj in range(NT):
                    blk = out_sb[:, jj, :]
                    if jj % 2 == 0:
                        nc.vector.tensor_scalar_mul(
                            blk, blk, gz[:, jj : jj + 1]
                        )
                    else:
                        nc.scalar.activation(
                            blk, blk, AF.Copy, scale=gz[:, jj : jj + 1]
                        )
                    qengs[jj % 3].dma_start(o_pt[:, jj, :], blk)

    nc.compile()
    return nc


def _get_nc():
    if "nc" not in _CACHE:
        _CACHE["nc"] = _build()
    return _CACHE["nc"]


def _in_maps(x, h):
    return [
        {
            "x": np.ascontiguousarray(x[:, c, :]),
            "h": np.ascontiguousarray(h[:, c, :]),
        }
        for c in range(NCORES)
    ]


def kernel(x, h):
    from concourse.bass_utils import run_bass_kernel_spmd

    x = np.asarray(x, dtype=np.float32)
    h = np.asarray(h, dtype=np.float32)
    assert x.shape == (N, B, E) and h.shape == (N, B, H)

    nc = _get_nc()
    res = run_bass_kernel_spmd(nc, _in_maps(x, h), core_ids=list(range(NCORES)))
    out = np.empty((N, B, H), dtype=np.float32)
    for c in range(NCORES):
        out[:, c, :] = res.results[c]["out"]
    return out


# Exposed for test.py: run once with tracing to get hardware exec time.
def run_traced(x, h):
    import os
    import shutil

    from concourse.bass_utils import run_bass_kernel_spmd

    x = np.asarray(x, dtype=np.float32)
    h = np.asarray(h, dtype=np.float32)
    nc = _get_nc()
    tdir = "/root/problem/trace_out"
    shutil.rmtree(tdir, ignore_errors=True)
    os.makedirs(tdir, exist_ok=True)
    res = run_bass_kernel_spmd(
        nc, _in_maps(x, h), core_ids=list(range(NCORES)), trace=True, tmpdir=tdir
    )
    out = np.empty((N, B, H), dtype=np.float32)
    for c in range(NCORES):
        out[:, c, :] = res.results[c]["out"]
    return out, res


# revision 20
# speedup vs baseline: 1.2243x; 1.0072x over previous
"""Trainium2 Bass kernel for nn_Attention_86663850099018.

Math (per batch b, reference semantics):
    xn = x_b / ||x_b rows||                      # (N, E) row-normalized
    S  = xn @ xn.T                               # (N, N) cosine scores, symmetric, |S|<=1
    P  = softmax(S, axis=1)                      # row softmax over keys
    U  = P @ h_b                                 # (N, H)
    out = U / frob_norm(U over all batches)      # the reference's H* factor cancels

Design notes:
  - Rows are relabeled p-major (row = p*16 + t) so DRAM<->SBUF moves are
    contiguous per partition.
  - Both big matmuls run in fp8e4 with perf_mode=DoubleRow, whose value is
    2x contraction per instruction (K=256/instr):
      * scores: one DR matmul per (row-block, col-chunk) contracts all of
        E=256. xn is pre-scaled by 16 (fp8-friendly range); exp applies
        the 1/256 correction via its scale argument.
      * E @ h: decomposed as U = colsum(h) + D @ h1 where D = exp(S) - 1
        is small (scores ~ N(0, 1/256)) and h1 = fp8(h). Quantization
        error of BOTH D and h enters only through the tiny D product
        (~0.2% each); the rank-1 colsum(h) term is computed exactly from
        the f32 h. DR pairs two row-tiles per instruction: 8 matmuls per
        output block instead of 16.
  - ACT table discipline: the only table-anchored function used is Exp
    (+Ln); Copy/Square are fillers present in every set. 1/sqrt(a) is
    computed as exp(-0.5*ln(a)) to stay inside natural_log_exp_and_others
    and avoid ~2.7us table switches per Sqrt.
  - Phase A (scores+exp+D) and phase B interleave per column chunk with a
    lag so the PE stream never waits on ACT/DVE; row sums (softmax denom)
    come free from exp's accum_out, sum-of-squares from Square accum_out.
    1/z and the global 1/norm fold into one writeback scale.
  - A dummy 4B AllReduce at kernel start warms the CC dispatch path and
    absorbs cross-core launch skew, halving the tail collective cost.
"""

import numpy as np

N, B, E, H = 2048, 8, 256, 512
P = 128
NT = N // P          # 16 row tiles / row blocks
EC = E // P          # 2 contraction chunks
SF = 512             # column-chunk width
NCH = N // SF        # 4 column chunks
TCH = 4              # tiles per input DMA chunk
NCORES = 8

_CACHE = {}


def _build():
    import concourse.mybir as mybir
    import concourse.tile as tile
    from concourse import bacc
    from concourse.masks import make_identity

    f32 = mybir.dt.float32
    f16 = mybir.dt.float16
    f8 = mybir.dt.float8e4
    AF = mybir.ActivationFunctionType
    ALU = mybir.AluOpType
    AX = mybir.AxisListType
    DR = mybir.MatmulPerfMode.DoubleRow

    nc = bacc.Bacc("TRN2", target_bir_lowering=False, debug=False, num_devices=NCORES)

    x_d = nc.dram_tensor("x", [N, E], f32, kind="ExternalInput").ap()
    h_d = nc.dram_tensor("h", [N, H], f32, kind="ExternalInput").ap()
    o_d = nc.dram_tensor("out", [N, H], f32, kind="ExternalOutput").ap()

    # p-major row relabeling: row = p*NT + t
    x_pt = x_d.rearrange("(p t) e -> p t e", t=NT)
    h_pt = h_d.rearrange("(p t) e -> p t e", t=NT)
    o_pt = o_d.rearrange("(p t) e -> p t e", t=NT)

    with tile.TileContext(nc) as tc:
        with (
            tc.tile_pool(name="const", bufs=1) as constp,
            tc.tile_pool(name="big", bufs=1) as bigp,
            tc.tile_pool(name="dramp", bufs=1, space="DRAM") as dramp,
        ):
            x_all = bigp.tile([P, NT, E], f32)
            h32 = bigp.tile([P, NT, H], f32)
            h1 = bigp.tile([P, NT, H], f8)        # fp8(h)
            xnt = bigp.tile([P, EC, N], f8)       # xn^T * 16, fp8
            d8 = bigp.tile([P, NT, N], f8)        # exp(S) - 1, fp8
            out_sb = bigp.tile([P, NT, H], f32)   # U_raw
            acc = bigp.tile([P, H], f32)          # running colsum of h
            acc16 = bigp.tile([P, H], f16)
            cs_bc = bigp.tile([P, SF], f32)       # colsum(h) broadcast
            ssqx = bigp.tile([P, NT], f32)
            lnv = bigp.tile([P, NT], f32)
            invn16 = bigp.tile([P, NT], f32)
            zpart = bigp.tile([P, NT * NCH], f32)
            zsum = bigp.tile([P, NT], f32)
            zinv = bigp.tile([P, NT], f32)
            zinv2 = bigp.tile([P, NT], f32)
            wss = bigp.tile([P, NT], f32)
            ssqraw = bigp.tile([P, NT], f32)
            ssqcol = bigp.tile([P, 1], f32)
            gz = bigp.tile([P, NT], f32)

            # ---------- input DMAs: 8 transfers over 3 HWDGE queues -------
            # ordered by first-need time of each chunk in the fused loop
            def xs(c):
                return (slice(None), slice(c * TCH, (c + 1) * TCH), slice(None))

            nc.sync.dma_start(x_all[xs(0)], x_pt[xs(0)])
            nc.scalar.dma_start(x_all[xs(1)], x_pt[xs(1)])
            nc.gpsimd.dma_start(x_all[xs(2)], x_pt[xs(2)])
            nc.sync.dma_start(x_all[xs(3)], x_pt[xs(3)])
            nc.scalar.dma_start(h32[:, 0:2, :], h_pt[:, 0:2, :])
            nc.gpsimd.dma_start(h32[:, 2:4, :], h_pt[:, 2:4, :])
            nc.gpsimd.dma_start(h32[xs(1)], h_pt[xs(1)])
            nc.sync.dma_start(h32[xs(2)], h_pt[xs(2)])
            nc.scalar.dma_start(h32[xs(3)], h_pt[xs(3)])

            ident = constp.tile([P, P], f16)
            make_identity(nc, ident[:])
            ones = constp.tile([P, 1], f32)
            nc.vector.memset(ones[:], 1.0)
            ones16 = constp.tile([P, 1], f16)
            nc.vector.memset(ones16[:], 1.0)
            zero1 = constp.tile([1, 1], f32)
            nc.vector.memset(zero1[:], 0.0)
            # preload both ACT table slots (exp -> sel0, ln -> sel1) while
            # the input DMAs are in flight, so the real Ln/Exp pair later
            # pays at most one reload
            dscr = constp.tile([1, 1], f32)
            nc.scalar.activation(dscr[:], ones[:1, :1], AF.Exp)
            nc.scalar.activation(dscr[:], ones[:1, :1], AF.Ln)

            # ---------- warmup collective (absorbs CC dispatch + skew) ----
            warm_in = dramp.tile([1, 1], f32)
            warm_out = dramp.tile([1, 1], f32, addr_space="Shared")
            nc.gpsimd.dma_start(warm_in[:], zero1[:])
            nc.gpsimd.collective_compute(
                "AllReduce",
                ALU.add,
                replica_groups=[list(range(NCORES))],
                ins=[warm_in.opt()],
                outs=[warm_out.opt()],
            )

            with (
                tc.tile_pool(name="ph0", bufs=3) as ph0,
                tc.tile_pool(name="sqp", bufs=2) as sqp,
                tc.tile_pool(name="escr", bufs=3) as escrp,
                tc.tile_pool(name="psT", bufs=2, space="PSUM") as psT,
                tc.tile_pool(name="psA", bufs=2, space="PSUM") as psAp,
                tc.tile_pool(name="psB", bufs=1, space="PSUM") as psBp,
            ):
                # phase 0: per-tile sum-of-squares on DVE as x chunks land,
                # then ONE batched invn16 = 16/||x|| = exp(-0.5*ln(ssq/256))
                # pair on ACT (Sqrt would thrash the activation tables).
                def sstt_ssq(t):
                    sqd = sqp.tile([P, E], f16, tag="sqd")
                    nc.vector.scalar_tensor_tensor(
                        sqd[:], x_all[:, t, :], 1.0, x_all[:, t, :],
                        ALU.mult, ALU.mult,
                        accum_out=ssqx[:, t : t + 1],
                    )

                def invn_batch(t0, t1):
                    nc.scalar.activation(
                        lnv[:, t0:t1], ssqx[:, t0:t1], AF.Ln, scale=1.0 / 256.0
                    )
                    nc.scalar.activation(
                        invn16[:, t0:t1], lnv[:, t0:t1], AF.Exp, scale=-0.5
                    )

                for t in range(12):
                    sstt_ssq(t)
                invn_batch(0, 12)

                # normalize + transpose one tile into fp8 xn^T
                def tile_finish(t):
                    xn = ph0.tile([P, E], f16, tag="xn")
                    nc.vector.tensor_scalar_mul(
                        xn[:], x_all[:, t, :], invn16[:, t : t + 1]
                    )
                    pt = psT.tile([P, EC, P], f16, tag="pt")
                    for cc in range(EC):
                        nc.tensor.transpose(
                            pt[:, cc, :], xn[:, cc * P : (cc + 1) * P],
                            ident[:],
                        )
                    nc.vector.tensor_copy(
                        xnt[:, :, t * P : (t + 1) * P], pt[:]
                    )

                for t in range(TCH):
                    tile_finish(t)

                # ---------- fused main loop over column chunks ------------
                for jc in range(NCH):
                    psBs = None
                    for s in range(NT + 3):
                        if s < NT:
                            i = s
                            ps = psAp.tile([P, SF], f32, tag="psA")
                            nc.tensor.matmul(
                                ps[:],
                                xnt[:, :, i * P : (i + 1) * P],
                                xnt[:, :, jc * SF : (jc + 1) * SF],
                                start=True,
                                stop=True,
                                perf_mode=DR,
                            )
                            if jc == 0:
                                nc.scalar.activation(
                                    h1[:, i, :], h32[:, i, :], AF.Copy
                                )
                            ee = escrp.tile([P, SF], f16, tag="ee")
                            nc.scalar.activation(
                                ee[:], ps[:], AF.Exp, scale=1.0 / 256.0
                            )
                            # d8 = E - 1; accum gives z_chunk - SF for free
                            nc.vector.tensor_scalar(
                                d8[:, i, jc * SF : (jc + 1) * SF],
                                ee[:],
                                -1.0,
                                1.0,
                                ALU.add,
                                ALU.mult,
                                accum_out=zpart[:, i * NCH + jc : i * NCH + jc + 1],
                            )
                            if jc == 0:
                                if i == 1:
                                    nc.vector.tensor_add(
                                        acc[:], h32[:, 0, :], h32[:, 1, :]
                                    )
                                elif i > 1:
                                    nc.vector.tensor_add(
                                        acc[:], acc[:], h32[:, i, :]
                                    )
                        if s >= 3 and (s - 3) % 2 == 0:
                            m = (s - 3) // 2
                            if m == 0:
                                psBs = [
                                    psBp.tile(
                                        [P, H], f32, name=f"psB{j}", tag=f"psB{j}"
                                    )
                                    for j in range(NCH)
                                ]
                            for j in range(NCH):
                                jj = jc * NCH + j
                                nc.tensor.matmul(
                                    psBs[j][:],
                                    d8[:, 2 * m : 2 * m + 2, jj * P : (jj + 1) * P],
                                    h1[:, 2 * m : 2 * m + 2, :],
                                    start=(m == 0),
                                    stop=(m == 7),
                                    perf_mode=DR,
                                )
                        if jc == 0:
                            if s == 0:
                                for t in range(12, NT):
                                    sstt_ssq(t)
                            elif s == 1:
                                invn_batch(12, NT)
                            if s + TCH < NT:
                                tile_finish(s + TCH)

                    if jc == 0:
                        # exact rank-1 colsum(h): partition-reduce the f32
                        # tile-tree sum via a single ones matmul
                        nc.vector.tensor_copy(acc16[:], acc[:])
                        psC = psAp.tile([P, SF], f32, name="psC", tag="psA")
                        nc.tensor.matmul(
                            psC[:1, :], ones16[:], acc16[:],
                            start=True, stop=True,
                        )
                        cs1 = constp.tile([1, SF], f32)
                        nc.scalar.copy(cs1[:], psC[:1, :])
                        nc.gpsimd.partition_broadcast(cs_bc[:], cs1[:])

                    for j in range(NCH):
                        jj = jc * NCH + j
                        nc.vector.tensor_add(
                            out_sb[:, jj, :], psBs[j][:], cs_bc[:]
                        )
                        sq2 = sqp.tile([P, H], f16, tag="sq2")
                        nc.scalar.activation(
                            sq2[:], out_sb[:, jj, :], AF.Square,
                            accum_out=ssqraw[:, jj : jj + 1],
                        )

            # ---------------- tail: global norm + writeback ---------------
            with (
                tc.tile_pool(name="tailp", bufs=2) as tailp,
                tc.tile_pool(name="psS", bufs=1, space="PSUM") as psS,
            ):
                nc.vector.tensor_reduce(
                    zsum[:],
                    zpart[:].rearrange("p (i j) -> p i j", j=NCH),
                    axis=AX.X,
                    op=ALU.add,
                )
                # zpart accumulated E-1, so add back the N ones per row
                nc.vector.tensor_scalar_add(zsum[:], zsum[:], float(N))
                nc.vector.reciprocal(zinv[:], zsum[:])
                nc.vector.tensor_mul(zinv2[:], zinv[:], zinv[:])
                nc.vector.tensor_mul(wss[:], zinv2[:], ssqraw[:])
                nc.vector.tensor_reduce(
                    ssqcol[:], wss[:], axis=AX.X, op=ALU.add
                )
                ps1 = psS.tile([1, 1], f32, tag="ps1")
                nc.tensor.matmul(ps1[:], ones[:], ssqcol[:], start=True, stop=True)
                ss11 = tailp.tile([1, 1], f32, tag="ss11")
                nc.scalar.copy(ss11[:], ps1[:])

                cc_in = dramp.tile([1, 1], f32)
                cc_out = dramp.tile([1, 1], f32, addr_space="Shared")
                nc.gpsimd.dma_start(cc_in[:], ss11[:])
                nc.gpsimd.collective_compute(
                    "AllReduce",
                    ALU.add,
                    replica_groups=[list(range(NCORES))],
                    ins=[cc_in.opt()],
                    outs=[cc_out.opt()],
                )
                agg = tailp.tile([1, 1], f32, tag="agg")
                nc.sync.dma_start(agg[:], cc_out[:])

                # ginv = 1/sqrt(agg) = exp(-0.5 ln(agg)); stays in-set
                lnag = tailp.tile([1, 1], f32, tag="lnag")
                ginv = tailp.tile([1, 1], f32, tag="ginv")
                nc.scalar.activation(lnag[:], agg[:], AF.Ln)
                nc.scalar.activation(ginv[:], lnag[:], AF.Exp, scale=-0.5)
                gbc = tailp.tile([P, 1], f32, tag="gbc")
                nc.gpsimd.partition_broadcast(gbc[:], ginv[:])
                nc.vector.tensor_scalar_mul(gz[:], zinv[:], gbc[:])

                # scale per block, then one big DMA per queue (a single
                # InstDMACopy fans out across all 16 SDMA engines)
                groups = [(0, 6, nc.sync), (6, 11, nc.scalar), (11, 16, nc.gpsimd)]
                for j0, j1, eng in groups:
                    for jj in range(j0, j1):
                        blk = out_sb[:, jj, :]
                        if jj % 2 == 0:
                            nc.vector.tensor_scalar_mul(
                                blk, blk, gz[:, jj : jj + 1]
                            )
                        else:
                            nc.scalar.activation(
                                blk, blk, AF.Copy, scale=gz[:, jj : jj + 1]
                            )
                    eng.dma_start(
                        o_pt[:, j0:j1, :], out_sb[:, j0:j1, :]
                    )

    nc.compile()
    return nc


def _get_nc():
    if "nc" not in _CACHE:
        _CACHE["nc"] = _build()
    return _CACHE["nc"]


def _in_maps(x, h):
    return [
        {
            "x": np.ascontiguousarray(x[:, c, :]),
            "h": np.ascontiguousarray(h[:, c, :]),
        }
        for c in range(NCORES)
    ]


def kernel(x, h):
    from concourse.bass_utils import run_bass_kernel_spmd

    x = np.asarray(x, dtype=np.float32)
    h = np.asarray(h, dtype=np.float32)
    assert x.shape == (N, B, E) and h.shape == (N, B, H)

    nc = _get_nc()
    res = run_bass_kernel_spmd(nc, _in_maps(x, h), core_ids=list(range(NCORES)))
    out = np.empty((N, B, H), dtype=np.float32)
    for c in range(NCORES):
        out[:, c, :] = res.results[c]["out"]
    return out


# Exposed for test.py: run once with tracing to get hardware exec time.
def run_traced(x, h):
    import os
    import shutil

    from concourse.bass_utils import run_bass_kernel_spmd

    x = np.asarray(x, dtype=np.float32)
    h = np.asarray(h, dtype=np.float32)
    nc = _get_nc()
    tdir = "/root/problem/trace_out"
    shutil.rmtree(tdir, ignore_errors=True)
    os.makedirs(tdir, exist_ok=True)
    res = run_bass_kernel_spmd(
        nc, _in_maps(x, h), core_ids=list(range(NCORES)), trace=True, tmpdir=tdir
    )
    out = np.empty((N, B, H), dtype=np.float32)
    for c in range(NCORES):
        out[:, c, :] = res.results[c]["out"]
    return out, res


# revision 23
# speedup vs baseline: 1.2780x; 1.0439x over previous
"""Trainium2 Bass kernel for nn_Attention_86663850099018.

Math (per batch b, reference semantics):
    xn = x_b / ||x_b rows||                      # (N, E) row-normalized
    S  = xn @ xn.T                               # (N, N) cosine scores, symmetric, |S|<=1
    P  = softmax(S, axis=1)                      # row softmax over keys
    U  = P @ h_b                                 # (N, H)
    out = U / frob_norm(U over all batches)      # the reference's H* factor cancels

Design notes:
  - Rows are relabeled p-major (row = p*16 + t) so DRAM<->SBUF moves are
    contiguous per partition.
  - Both big matmuls run in fp8e4 with perf_mode=DoubleRow, whose value is
    2x contraction per instruction (K=256/instr):
      * scores: one DR matmul per (row-block, col-chunk) contracts all of
        E=256. xn is pre-scaled by 16 (fp8-friendly range); exp applies
        the 1/256 correction via its scale argument.
      * E @ h: decomposed as U = colsum(h) + D @ h1 where D = exp(S) - 1
        is small (scores ~ N(0, 1/256)) and h1 = fp8(h). Quantization
        error of BOTH D and h enters only through the tiny D product
        (~0.2% each); the rank-1 colsum(h) term is computed exactly from
        the f32 h. DR pairs two row-tiles per instruction: 8 matmuls per
        output block instead of 16.
  - ACT table discipline: the only table-anchored function used is Exp
    (+Ln); Copy/Square are fillers present in every set. 1/sqrt(a) is
    computed as exp(-0.5*ln(a)) to stay inside natural_log_exp_and_others
    and avoid ~2.7us table switches per Sqrt.
  - Phase A (scores+exp+D) and phase B interleave per column chunk with a
    lag so the PE stream never waits on ACT/DVE; row sums (softmax denom)
    come free from exp's accum_out, sum-of-squares from Square accum_out.
    1/z and the global 1/norm fold into one writeback scale.
  - A dummy 4B AllReduce at kernel start warms the CC dispatch path and
    absorbs cross-core launch skew, halving the tail collective cost.
"""

import numpy as np

N, B, E, H = 2048, 8, 256, 512
P = 128
NT = N // P          # 16 row tiles / row blocks
EC = E // P          # 2 contraction chunks
SF = 512             # column-chunk width
NCH = N // SF        # 4 column chunks
TCH = 4              # tiles per input DMA chunk
NCORES = 8

_CACHE = {}


def _build():
    import concourse.mybir as mybir
    import concourse.tile as tile
    from concourse import bacc
    from concourse.masks import make_identity

    f32 = mybir.dt.float32
    f16 = mybir.dt.float16
    f8 = mybir.dt.float8e4
    AF = mybir.ActivationFunctionType
    ALU = mybir.AluOpType
    AX = mybir.AxisListType
    DR = mybir.MatmulPerfMode.DoubleRow

    nc = bacc.Bacc("TRN2", target_bir_lowering=False, debug=False, num_devices=NCORES)

    x_d = nc.dram_tensor("x", [N, E], f32, kind="ExternalInput").ap()
    h_d = nc.dram_tensor("h", [N, H], f32, kind="ExternalInput").ap()
    o_d = nc.dram_tensor("out", [N, H], f32, kind="ExternalOutput").ap()

    # p-major row relabeling: row = p*NT + t
    x_pt = x_d.rearrange("(p t) e -> p t e", t=NT)
    h_pt = h_d.rearrange("(p t) e -> p t e", t=NT)
    o_pt = o_d.rearrange("(p t) e -> p t e", t=NT)

    with tile.TileContext(nc) as tc:
        with (
            tc.tile_pool(name="const", bufs=1) as constp,
            tc.tile_pool(name="big", bufs=1) as bigp,
            tc.tile_pool(name="dramp", bufs=1, space="DRAM") as dramp,
        ):
            x_all = bigp.tile([P, NT, E], f32)
            h32 = bigp.tile([P, NT, H], f32)
            h1 = bigp.tile([P, NT, H], f8)        # fp8(h)
            xnt = bigp.tile([P, EC, N], f8)       # xn^T * 16, fp8
            d8 = bigp.tile([P, NT, N], f8)        # exp(S) - 1, fp8
            out_sb = bigp.tile([P, NT, H], f32)   # U_raw
            acc = bigp.tile([P, H], f32)          # running colsum of h
            acc16 = bigp.tile([P, H], f16)
            cs_bc = bigp.tile([P, SF], f32)       # colsum(h) broadcast
            ssqx = bigp.tile([P, NT], f32)
            lnv = bigp.tile([P, NT], f32)
            invn16 = bigp.tile([P, NT], f32)
            zpart = bigp.tile([P, NT * NCH], f32)
            zsum = bigp.tile([P, NT], f32)
            zinv = bigp.tile([P, NT], f32)
            zinv2 = bigp.tile([P, NT], f32)
            wss = bigp.tile([P, NT], f32)
            ssqraw = bigp.tile([P, NT], f32)
            ssqcol = bigp.tile([P, 1], f32)
            gz = bigp.tile([P, NT], f32)

            # ---------- input DMAs: 8 transfers over 3 HWDGE queues -------
            # ordered by first-need time of each chunk in the fused loop
            def xs(c):
                return (slice(None), slice(c * TCH, (c + 1) * TCH), slice(None))

            nc.sync.dma_start(x_all[xs(0)], x_pt[xs(0)])
            nc.scalar.dma_start(x_all[xs(1)], x_pt[xs(1)])
            nc.gpsimd.dma_start(x_all[xs(2)], x_pt[xs(2)])
            nc.sync.dma_start(x_all[xs(3)], x_pt[xs(3)])
            nc.scalar.dma_start(h32[:, 0:2, :], h_pt[:, 0:2, :])
            nc.gpsimd.dma_start(h32[:, 2:4, :], h_pt[:, 2:4, :])
            nc.gpsimd.dma_start(h32[xs(1)], h_pt[xs(1)])
            nc.sync.dma_start(h32[xs(2)], h_pt[xs(2)])
            nc.scalar.dma_start(h32[xs(3)], h_pt[xs(3)])

            ident = constp.tile([P, P], f16)
            make_identity(nc, ident[:])
            ones = constp.tile([P, 1], f32)
            nc.vector.memset(ones[:], 1.0)
            ones16 = constp.tile([P, 1], f16)
            nc.vector.memset(ones16[:], 1.0)
            zero1 = constp.tile([1, 1], f32)
            nc.vector.memset(zero1[:], 0.0)
            # preload the rsqrt table set while the input DMAs are in
            # flight, so the real invn ops below don't pay the ~2.7us load
            dscr = constp.tile([1, 1], f32)
            nc.scalar.activation(dscr[:], ones[:1, :1], AF.Abs_reciprocal_sqrt)

            # ---------- warmup collective (absorbs CC dispatch + skew) ----
            warm_in = dramp.tile([1, 1], f32)
            warm_out = dramp.tile([1, 1], f32, addr_space="Shared")
            nc.gpsimd.dma_start(warm_in[:], zero1[:])
            nc.gpsimd.collective_compute(
                "AllReduce",
                ALU.add,
                replica_groups=[list(range(NCORES))],
                ins=[warm_in.opt()],
                outs=[warm_out.opt()],
            )

            with (
                tc.tile_pool(name="ph0", bufs=3) as ph0,
                tc.tile_pool(name="sqp", bufs=2) as sqp,
                tc.tile_pool(name="escr", bufs=3) as escrp,
                tc.tile_pool(name="psT", bufs=2, space="PSUM") as psT,
                tc.tile_pool(name="psA", bufs=2, space="PSUM") as psAp,
                tc.tile_pool(name="psB", bufs=1, space="PSUM") as psBp,
            ):
                # phase 0: per-tile sum-of-squares on DVE as x chunks land,
                # then ONE batched invn16 = 16/||x|| = exp(-0.5*ln(ssq/256))
                # pair on ACT (Sqrt would thrash the activation tables).
                def sstt_ssq(t):
                    sqd = sqp.tile([P, E], f16, tag="sqd")
                    nc.vector.scalar_tensor_tensor(
                        sqd[:], x_all[:, t, :], 1.0, x_all[:, t, :],
                        ALU.mult, ALU.mult,
                        accum_out=ssqx[:, t : t + 1],
                    )

                def invn_batch(t0, t1):
                    # invn16 = 16/||x|| = 1/sqrt(ssq/256), one table set
                    nc.scalar.activation(
                        invn16[:, t0:t1], ssqx[:, t0:t1],
                        AF.Abs_reciprocal_sqrt, scale=1.0 / 256.0,
                    )

                for t in range(12):
                    sstt_ssq(t)
                invn_batch(0, 12)

                # normalize + transpose one tile into fp8 xn^T
                def tile_finish(t):
                    xn = ph0.tile([P, E], f16, tag="xn")
                    nc.vector.tensor_scalar_mul(
                        xn[:], x_all[:, t, :], invn16[:, t : t + 1]
                    )
                    pt = psT.tile([P, EC, P], f16, tag="pt")
                    for cc in range(EC):
                        nc.tensor.transpose(
                            pt[:, cc, :], xn[:, cc * P : (cc + 1) * P],
                            ident[:],
                        )
                    nc.vector.tensor_copy(
                        xnt[:, :, t * P : (t + 1) * P], pt[:]
                    )

                for t in range(TCH):
                    tile_finish(t)

                # ---------- fused main loop over column chunks ------------
                for jc in range(NCH):
                    psBs = None
                    for s in range(NT + 3):
                        if s < NT:
                            i = s
                            ps = psAp.tile([P, SF], f32, tag="psA")
                            nc.tensor.matmul(
                                ps[:],
                                xnt[:, :, i * P : (i + 1) * P],
                                xnt[:, :, jc * SF : (jc + 1) * SF],
                                start=True,
                                stop=True,
                                perf_mode=DR,
                            )
                            if jc == 0:
                                nc.scalar.activation(
                                    h1[:, i, :], h32[:, i, :], AF.Copy
                                )
                            ee = escrp.tile([P, SF], f16, tag="ee")
                            nc.scalar.activation(
                                ee[:], ps[:], AF.Exp, scale=1.0 / 256.0
                            )
                            # d8 = E - 1; accum gives z_chunk - SF for free
                            nc.vector.tensor_scalar(
                                d8[:, i, jc * SF : (jc + 1) * SF],
                                ee[:],
                                -1.0,
                                1.0,
                                ALU.add,
                                ALU.mult,
                                accum_out=zpart[:, i * NCH + jc : i * NCH + jc + 1],
                            )
                            if jc == 0:
                                if i == 1:
                                    nc.vector.tensor_add(
                                        acc[:], h32[:, 0, :], h32[:, 1, :]
                                    )
                                elif i > 1:
                                    nc.vector.tensor_add(
                                        acc[:], acc[:], h32[:, i, :]
                                    )
                        if s >= 3 and (s - 3) % 2 == 0:
                            m = (s - 3) // 2
                            if m == 0:
                                psBs = [
                                    psBp.tile(
                                        [P, H], f32, name=f"psB{j}", tag=f"psB{j}"
                                    )
                                    for j in range(NCH)
                                ]
                            for j in range(NCH):
                                jj = jc * NCH + j
                                nc.tensor.matmul(
                                    psBs[j][:],
                                    d8[:, 2 * m : 2 * m + 2, jj * P : (jj + 1) * P],
                                    h1[:, 2 * m : 2 * m + 2, :],
                                    start=(m == 0),
                                    stop=(m == 7),
                                    perf_mode=DR,
                                )
                        if jc == 0:
                            if s == 0:
                                for t in range(12, NT):
                                    sstt_ssq(t)
                            elif s == 1:
                                invn_batch(12, NT)
                            if s + TCH < NT:
                                tile_finish(s + TCH)

                    if jc == 0:
                        # exact rank-1 colsum(h): partition-reduce the f32
                        # tile-tree sum via a single ones matmul
                        nc.vector.tensor_copy(acc16[:], acc[:])
                        psC = psAp.tile([P, SF], f32, name="psC", tag="psA")
                        nc.tensor.matmul(
                            psC[:1, :], ones16[:], acc16[:],
                            start=True, stop=True,
                        )
                        cs1 = constp.tile([1, SF], f32)
                        nc.scalar.copy(cs1[:], psC[:1, :])
                        nc.gpsimd.partition_broadcast(cs_bc[:], cs1[:])

                    for j in range(NCH):
                        jj = jc * NCH + j
                        nc.vector.tensor_add(
                            out_sb[:, jj, :], psBs[j][:], cs_bc[:]
                        )
                        sq2 = sqp.tile([P, H], f16, tag="sq2")
                        nc.scalar.activation(
                            sq2[:], out_sb[:, jj, :], AF.Square,
                            accum_out=ssqraw[:, jj : jj + 1],
                        )

            # ---------------- tail: global norm + writeback ---------------
            with (
                tc.tile_pool(name="tailp", bufs=2) as tailp,
                tc.tile_pool(name="psS", bufs=1, space="PSUM") as psS,
            ):
                nc.vector.tensor_reduce(
                    zsum[:],
                    zpart[:].rearrange("p (i j) -> p i j", j=NCH),
                    axis=AX.X,
                    op=ALU.add,
                )
                # zpart accumulated E-1, so add back the N ones per row
                nc.vector.tensor_scalar_add(zsum[:], zsum[:], float(N))
                nc.vector.reciprocal(zinv[:], zsum[:])
                nc.vector.tensor_mul(zinv2[:], zinv[:], zinv[:])
                nc.vector.tensor_mul(wss[:], zinv2[:], ssqraw[:])
                nc.vector.tensor_reduce(
                    ssqcol[:], wss[:], axis=AX.X, op=ALU.add
                )
                ps1 = psS.tile([1, 1], f32, tag="ps1")
                nc.tensor.matmul(ps1[:], ones[:], ssqcol[:], start=True, stop=True)
                ss11 = tailp.tile([1, 1], f32, tag="ss11")
                nc.scalar.copy(ss11[:], ps1[:])

                cc_in = dramp.tile([1, 1], f32)
                cc_out = dramp.tile([1, 1], f32, addr_space="Shared")
                nc.gpsimd.dma_start(cc_in[:], ss11[:])
                nc.gpsimd.collective_compute(
                    "AllReduce",
                    ALU.add,
                    replica_groups=[list(range(NCORES))],
                    ins=[cc_in.opt()],
                    outs=[cc_out.opt()],
                )
                agg = tailp.tile([1, 1], f32, tag="agg")
                nc.sync.dma_start(agg[:], cc_out[:])

                # while the collective is in flight, pre-scale U by 1/z so
                # only the uniform global factor remains afterwards
                for jj in range(NT):
                    blk = out_sb[:, jj, :]
                    if jj % 2 == 0:
                        nc.vector.tensor_scalar_mul(
                            blk, blk, zinv[:, jj : jj + 1]
                        )
                    else:
                        nc.scalar.activation(
                            blk, blk, AF.Copy, scale=zinv[:, jj : jj + 1]
                        )

                ginv = tailp.tile([1, 1], f32, tag="ginv")
                nc.scalar.activation(ginv[:], agg[:], AF.Abs_reciprocal_sqrt)
                gbc = tailp.tile([P, 1], f32, tag="gbc")
                nc.gpsimd.partition_broadcast(gbc[:], ginv[:])

                # uniform 1/gnorm scale split DVE/ACT per group, then one
                # big DMA per queue (a single InstDMACopy fans out across
                # all 16 SDMA engines)
                groups = [(0, 6, nc.sync), (6, 11, nc.scalar), (11, 16, nc.gpsimd)]
                for j0, j1, eng in groups:
                    jm = (j0 + j1) // 2
                    nc.vector.tensor_scalar_mul(
                        out_sb[:, j0:jm, :], out_sb[:, j0:jm, :], gbc[:]
                    )
                    nc.scalar.activation(
                        out_sb[:, jm:j1, :], out_sb[:, jm:j1, :],
                        AF.Copy, scale=gbc[:],
                    )
                    eng.dma_start(
                        o_pt[:, j0:j1, :], out_sb[:, j0:j1, :]
                    )

    nc.compile()
    return nc


def _get_nc():
    if "nc" not in _CACHE:
        _CACHE["nc"] = _build()
    return _CACHE["nc"]


def _in_maps(x, h):
    return [
        {
            "x": np.ascontiguousarray(x[:, c, :]),
            "h": np.ascontiguousarray(h[:, c, :]),
        }
        for c in range(NCORES)
    ]


def kernel(x, h):
    from concourse.bass_utils import run_bass_kernel_spmd

    x = np.asarray(x, dtype=np.float32)
    h = np.asarray(h, dtype=np.float32)
    assert x.shape == (N, B, E) and h.shape == (N, B, H)

    nc = _get_nc()
    res = run_bass_kernel_spmd(nc, _in_maps(x, h), core_ids=list(range(NCORES)))
    out = np.empty((N, B, H), dtype=np.float32)
    for c in range(NCORES):
        out[:, c, :] = res.results[c]["out"]
    return out


# Exposed for test.py: run once with tracing to get hardware exec time.
def run_traced(x, h):
    import os
    import shutil

    from concourse.bass_utils import run_bass_kernel_spmd

    x = np.asarray(x, dtype=np.float32)
    h = np.asarray(h, dtype=np.float32)
    nc = _get_nc()
    tdir = "/root/problem/trace_out"
    shutil.rmtree(tdir, ignore_errors=True)
    os.makedirs(tdir, exist_ok=True)
    res = run_bass_kernel_spmd(
        nc, _in_maps(x, h), core_ids=list(range(NCORES)), trace=True, tmpdir=tdir
    )
    out = np.empty((N, B, H), dtype=np.float32)
    for c in range(NCORES):
        out[:, c, :] = res.results[c]["out"]
    return out, res


# revision 26
# speedup vs baseline: 1.3463x; 1.0535x over previous
"""Trainium2 Bass kernel for nn_Attention_86663850099018.

Math (per batch b, reference semantics):
    xn = x_b / ||x_b rows||                      # (N, E) row-normalized
    S  = xn @ xn.T                               # (N, N) cosine scores, symmetric, |S|<=1
    P  = softmax(S, axis=1)                      # row softmax over keys
    U  = P @ h_b                                 # (N, H)
    out = U / frob_norm(U over all batches)      # the reference's H* factor cancels

Design notes:
  - Rows are relabeled p-major (row = p*16 + t) so DRAM<->SBUF moves are
    contiguous per partition.
  - Both big matmuls run in fp8e4 with perf_mode=DoubleRow, whose value is
    2x contraction per instruction (K=256/instr):
      * scores: one DR matmul per (row-block, col-chunk) contracts all of
        E=256. xn is pre-scaled by 16 (fp8-friendly range); exp applies
        the 1/256 correction via its scale argument.
      * E @ h: decomposed as U = colsum(h) + D @ h1 where D = exp(S) - 1
        is small (scores ~ N(0, 1/256)) and h1 = fp8(h). Quantization
        error of BOTH D and h enters only through the tiny D product
        (~0.2% each); the rank-1 colsum(h) term is computed exactly from
        the f32 h. DR pairs two row-tiles per instruction: 8 matmuls per
        output block instead of 16.
  - ACT table discipline: the only table-anchored function used is Exp
    (+Ln); Copy/Square are fillers present in every set. 1/sqrt(a) is
    computed as exp(-0.5*ln(a)) to stay inside natural_log_exp_and_others
    and avoid ~2.7us table switches per Sqrt.
  - Phase A (scores+exp+D) and phase B interleave per column chunk with a
    lag so the PE stream never waits on ACT/DVE; row sums (softmax denom)
    come free from exp's accum_out, sum-of-squares from Square accum_out.
    1/z and the global 1/norm fold into one writeback scale.
  - A dummy 4B AllReduce at kernel start warms the CC dispatch path and
    absorbs cross-core launch skew, halving the tail collective cost.
"""

import numpy as np

N, B, E, H = 2048, 8, 256, 512
P = 128
NT = N // P          # 16 row tiles / row blocks
EC = E // P          # 2 contraction chunks
SF = 512             # column-chunk width
NCH = N // SF        # 4 column chunks
TCH = 4              # tiles per input DMA chunk
NCORES = 8

_CACHE = {}


def _build():
    import concourse.mybir as mybir
    import concourse.tile as tile
    from concourse import bacc
    from concourse.masks import make_identity

    f32 = mybir.dt.float32
    f16 = mybir.dt.float16
    f8 = mybir.dt.float8e4
    AF = mybir.ActivationFunctionType
    ALU = mybir.AluOpType
    AX = mybir.AxisListType
    DR = mybir.MatmulPerfMode.DoubleRow

    nc = bacc.Bacc("TRN2", target_bir_lowering=False, debug=False, num_devices=NCORES)

    x_d = nc.dram_tensor("x", [N, E], f32, kind="ExternalInput").ap()
    h_d = nc.dram_tensor("h", [N, H], f32, kind="ExternalInput").ap()
    o_d = nc.dram_tensor("out", [N, H], f32, kind="ExternalOutput").ap()

    # p-major row relabeling: row = p*NT + t
    x_pt = x_d.rearrange("(p t) e -> p t e", t=NT)
    h_pt = h_d.rearrange("(p t) e -> p t e", t=NT)
    o_pt = o_d.rearrange("(p t) e -> p t e", t=NT)

    with tile.TileContext(nc) as tc:
        with (
            tc.tile_pool(name="const", bufs=1) as constp,
            tc.tile_pool(name="big", bufs=1) as bigp,
            tc.tile_pool(name="dramp", bufs=1, space="DRAM") as dramp,
        ):
            x_all = bigp.tile([P, NT, E], f32)
            h32 = bigp.tile([P, NT, H], f32)
            h1 = bigp.tile([P, NT, H], f8)        # fp8(h)
            xnt = bigp.tile([P, EC, N], f8)       # xn^T * 16, fp8
            d8 = bigp.tile([P, NT, N], f8)        # exp(S) - 1, fp8
            out_sb = bigp.tile([P, NT, H], f32)   # U_raw
            acc = bigp.tile([P, H], f32)          # running colsum of h
            acc16 = bigp.tile([P, H], f16)
            cs_bc = bigp.tile([P, SF], f32)       # colsum(h) broadcast
            ssqx = bigp.tile([P, NT], f32)
            lnv = bigp.tile([P, NT], f32)
            invn16 = bigp.tile([P, NT], f32)
            zpart = bigp.tile([P, NT * NCH], f32)
            zsum = bigp.tile([P, NT], f32)
            zinv = bigp.tile([P, NT], f32)
            zinv2 = bigp.tile([P, NT], f32)
            wss = bigp.tile([P, NT], f32)
            ssqraw = bigp.tile([P, NT], f32)
            ssqcol = bigp.tile([P, 1], f32)
            gz = bigp.tile([P, NT], f32)

            # ---------- input DMAs: 8 transfers over 3 HWDGE queues -------
            # ordered by first-need time of each chunk in the fused loop
            def xs(c):
                return (slice(None), slice(c * TCH, (c + 1) * TCH), slice(None))

            nc.sync.dma_start(x_all[xs(0)], x_pt[xs(0)])
            nc.scalar.dma_start(x_all[xs(1)], x_pt[xs(1)])
            nc.gpsimd.dma_start(x_all[xs(2)], x_pt[xs(2)])
            nc.sync.dma_start(x_all[xs(3)], x_pt[xs(3)])
            nc.scalar.dma_start(h32[:, 0:2, :], h_pt[:, 0:2, :])
            nc.gpsimd.dma_start(h32[:, 2:4, :], h_pt[:, 2:4, :])
            nc.gpsimd.dma_start(h32[xs(1)], h_pt[xs(1)])
            nc.sync.dma_start(h32[xs(2)], h_pt[xs(2)])
            nc.scalar.dma_start(h32[xs(3)], h_pt[xs(3)])

            ident = constp.tile([P, P], f16)
            make_identity(nc, ident[:])
            ones = constp.tile([P, 1], f32)
            nc.vector.memset(ones[:], 1.0)
            ones16 = constp.tile([P, 1], f16)
            nc.vector.memset(ones16[:], 1.0)
            zero1 = constp.tile([1, 1], f32)
            nc.vector.memset(zero1[:], 0.0)
            # preload the rsqrt table set while the input DMAs are in
            # flight, so the real invn ops below don't pay the ~2.7us load
            dscr = constp.tile([1, 1], f32)
            nc.scalar.activation(dscr[:], ones[:1, :1], AF.Abs_reciprocal_sqrt)

            # ---------- warmup collective (absorbs CC dispatch + skew) ----
            warm_in = dramp.tile([1, 1], f32)
            warm_out = dramp.tile([1, 1], f32, addr_space="Shared")
            nc.gpsimd.dma_start(warm_in[:], zero1[:])
            nc.gpsimd.collective_compute(
                "AllReduce",
                ALU.add,
                replica_groups=[list(range(NCORES))],
                ins=[warm_in.opt()],
                outs=[warm_out.opt()],
            )

            with (
                tc.tile_pool(name="ph0", bufs=3) as ph0,
                tc.tile_pool(name="sqp", bufs=2) as sqp,
                tc.tile_pool(name="escr", bufs=3) as escrp,
                tc.tile_pool(name="psT", bufs=2, space="PSUM") as psT,
                tc.tile_pool(name="psA", bufs=2, space="PSUM") as psAp,
                tc.tile_pool(name="psB", bufs=1, space="PSUM") as psBp,
            ):
                # phase 0: per-tile sum-of-squares on DVE as x chunks land,
                # then ONE batched invn16 = 16/||x|| = exp(-0.5*ln(ssq/256))
                # pair on ACT (Sqrt would thrash the activation tables).
                def sstt_ssq(t):
                    sqd = sqp.tile([P, E], f16, tag="sqd")
                    nc.vector.scalar_tensor_tensor(
                        sqd[:], x_all[:, t, :], 1.0, x_all[:, t, :],
                        ALU.mult, ALU.mult,
                        accum_out=ssqx[:, t : t + 1],
                    )

                def invn_batch(t0, t1):
                    # invn16 = 16/||x|| = 1/sqrt(ssq/256), one table set
                    nc.scalar.activation(
                        invn16[:, t0:t1], ssqx[:, t0:t1],
                        AF.Abs_reciprocal_sqrt, scale=1.0 / 256.0,
                    )

                # two batches so tiles 0-7 (x chunks 0-1) unblock the PE
                # early; both ARS ops precede the first Exp so the exp
                # table set loads exactly once
                for t in range(8):
                    sstt_ssq(t)
                invn_batch(0, 8)

                # normalize + transpose one tile into fp8 xn^T
                def tile_finish(t):
                    xn = ph0.tile([P, E], f16, tag="xn")
                    nc.vector.tensor_scalar_mul(
                        xn[:], x_all[:, t, :], invn16[:, t : t + 1]
                    )
                    pt = psT.tile([P, EC, P], f16, tag="pt")
                    for cc in range(EC):
                        nc.tensor.transpose(
                            pt[:, cc, :], xn[:, cc * P : (cc + 1) * P],
                            ident[:],
                        )
                    nc.vector.tensor_copy(
                        xnt[:, :, t * P : (t + 1) * P], pt[:]
                    )

                for t in range(TCH):
                    tile_finish(t)
                for t in range(8, NT):
                    sstt_ssq(t)
                invn_batch(8, NT)

                # ---------- fused main loop over column chunks ------------
                for jc in range(NCH):
                    psBs = None
                    for s in range(NT + 3):
                        if s < NT:
                            i = s
                            ps = psAp.tile([P, SF], f32, tag="psA")
                            nc.tensor.matmul(
                                ps[:],
                                xnt[:, :, i * P : (i + 1) * P],
                                xnt[:, :, jc * SF : (jc + 1) * SF],
                                start=True,
                                stop=True,
                                perf_mode=DR,
                            )
                            if jc == 0:
                                nc.scalar.activation(
                                    h1[:, i, :], h32[:, i, :], AF.Copy
                                )
                            ee = escrp.tile([P, SF], f16, tag="ee")
                            nc.scalar.activation(
                                ee[:], ps[:], AF.Exp, scale=1.0 / 256.0
                            )
                            # d8 = E - 1; accum gives z_chunk - SF for free
                            nc.vector.tensor_scalar(
                                d8[:, i, jc * SF : (jc + 1) * SF],
                                ee[:],
                                -1.0,
                                1.0,
                                ALU.add,
                                ALU.mult,
                                accum_out=zpart[:, i * NCH + jc : i * NCH + jc + 1],
                            )
                            if jc == 0:
                                if i == 1:
                                    nc.vector.tensor_add(
                                        acc[:], h32[:, 0, :], h32[:, 1, :]
                                    )
                                elif i > 1:
                                    nc.vector.tensor_add(
                                        acc[:], acc[:], h32[:, i, :]
                                    )
                        if s >= 3 and (s - 3) % 2 == 0:
                            m = (s - 3) // 2
                            if m == 0:
                                psBs = [
                                    psBp.tile(
                                        [P, H], f32, name=f"psB{j}", tag=f"psB{j}"
                                    )
                                    for j in range(NCH)
                                ]
                            for j in range(NCH):
                                jj = jc * NCH + j
                                nc.tensor.matmul(
                                    psBs[j][:],
                                    d8[:, 2 * m : 2 * m + 2, jj * P : (jj + 1) * P],
                                    h1[:, 2 * m : 2 * m + 2, :],
                                    start=(m == 0),
                                    stop=(m == 7),
                                    perf_mode=DR,
                                )
                        if jc == 0 and s + TCH < NT:
                            tile_finish(s + TCH)

                    if jc == 0:
                        # exact rank-1 colsum(h): partition-reduce the f32
                        # tile-tree sum via a single ones matmul
                        nc.vector.tensor_copy(acc16[:], acc[:])
                        psC = psAp.tile([P, SF], f32, name="psC", tag="psA")
                        nc.tensor.matmul(
                            psC[:1, :], ones16[:], acc16[:],
                            start=True, stop=True,
                        )
                        cs1 = constp.tile([1, SF], f32)
                        nc.scalar.copy(cs1[:], psC[:1, :])
                        nc.gpsimd.partition_broadcast(cs_bc[:], cs1[:])

                    for j in range(NCH):
                        jj = jc * NCH + j
                        nc.vector.tensor_add(
                            out_sb[:, jj, :], psBs[j][:], cs_bc[:]
                        )
                        sq2 = sqp.tile([P, H], f16, tag="sq2")
                        nc.scalar.activation(
                            sq2[:], out_sb[:, jj, :], AF.Square,
                            accum_out=ssqraw[:, jj : jj + 1],
                        )

            # ---------------- tail: global norm + writeback ---------------
            with (
                tc.tile_pool(name="tailp", bufs=2) as tailp,
                tc.tile_pool(name="psS", bufs=1, space="PSUM") as psS,
            ):
                nc.vector.tensor_reduce(
                    zsum[:],
                    zpart[:].rearrange("p (i j) -> p i j", j=NCH),
                    axis=AX.X,
                    op=ALU.add,
                )
                # zpart accumulated E-1, so add back the N ones per row
                nc.vector.tensor_scalar_add(zsum[:], zsum[:], float(N))
                nc.vector.reciprocal(zinv[:], zsum[:])
                nc.vector.tensor_mul(zinv2[:], zinv[:], zinv[:])
                nc.vector.tensor_mul(wss[:], zinv2[:], ssqraw[:])
                nc.vector.tensor_reduce(
                    ssqcol[:], wss[:], axis=AX.X, op=ALU.add
                )
                ps1 = psS.tile([1, 1], f32, tag="ps1")
                nc.tensor.matmul(ps1[:], ones[:], ssqcol[:], start=True, stop=True)
                ss11 = tailp.tile([1, 1], f32, tag="ss11")
                nc.scalar.copy(ss11[:], ps1[:])

                cc_in = dramp.tile([1, 1], f32)
                cc_out = dramp.tile([1, 1], f32, addr_space="Shared")
                nc.gpsimd.dma_start(cc_in[:], ss11[:])
                nc.gpsimd.collective_compute(
                    "AllReduce",
                    ALU.add,
                    replica_groups=[list(range(NCORES))],
                    ins=[cc_in.opt()],
                    outs=[cc_out.opt()],
                )
                agg = tailp.tile([1, 1], f32, tag="agg")
                nc.sync.dma_start(agg[:], cc_out[:])

                # while the collective is in flight, pre-scale U by 1/z so
                # only the uniform global factor remains afterwards
                for jj in range(NT):
                    blk = out_sb[:, jj, :]
                    if jj % 2 == 0:
                        nc.vector.tensor_scalar_mul(
                            blk, blk, zinv[:, jj : jj + 1]
                        )
                    else:
                        nc.scalar.activation(
                            blk, blk, AF.Copy, scale=zinv[:, jj : jj + 1]
                        )

                ginv = tailp.tile([1, 1], f32, tag="ginv")
                nc.scalar.activation(ginv[:], agg[:], AF.Abs_reciprocal_sqrt)
                gbc = tailp.tile([P, 1], f32, tag="gbc")
                nc.gpsimd.partition_broadcast(gbc[:], ginv[:])

                # uniform 1/gnorm scale split DVE/ACT per group, then one
                # big DMA per queue (a single InstDMACopy fans out across
                # all 16 SDMA engines)
                groups = [(0, 6, nc.sync), (6, 11, nc.scalar), (11, 16, nc.gpsimd)]
                for j0, j1, eng in groups:
                    jm = (j0 + j1) // 2
                    nc.vector.tensor_scalar_mul(
                        out_sb[:, j0:jm, :], out_sb[:, j0:jm, :], gbc[:]
                    )
                    nc.scalar.activation(
                        out_sb[:, jm:j1, :], out_sb[:, jm:j1, :],
                        AF.Copy, scale=gbc[:],
                    )
                    eng.dma_start(
                        o_pt[:, j0:j1, :], out_sb[:, j0:j1, :]
                    )

    nc.compile()
    return nc


def _get_nc():
    if "nc" not in _CACHE:
        _CACHE["nc"] = _build()
    return _CACHE["nc"]


def _in_maps(x, h):
    return [
        {
            "x": np.ascontiguousarray(x[:, c, :]),
            "h": np.ascontiguousarray(h[:, c, :]),
        }
        for c in range(NCORES)
    ]


def kernel(x, h):
    from concourse.bass_utils import run_bass_kernel_spmd

    x = np.asarray(x, dtype=np.float32)
    h = np.asarray(h, dtype=np.float32)
    assert x.shape == (N, B, E) and h.shape == (N, B, H)

    nc = _get_nc()
    res = run_bass_kernel_spmd(nc, _in_maps(x, h), core_ids=list(range(NCORES)))
    out = np.empty((N, B, H), dtype=np.float32)
    for c in range(NCORES):
        out[:, c, :] = res.results[c]["out"]
    return out


# Exposed for test.py: run once with tracing to get hardware exec time.
def run_traced(x, h):
    import os
    import shutil

    from concourse.bass_utils import run_bass_kernel_spmd

    x = np.asarray(x, dtype=np.float32)
    h = np.asarray(h, dtype=np.float32)
    nc = _get_nc()
    tdir = "/root/problem/trace_out"
    shutil.rmtree(tdir, ignore_errors=True)
    os.makedirs(tdir, exist_ok=True)
    res = run_bass_kernel_spmd(
        nc, _in_maps(x, h), core_ids=list(range(NCORES)), trace=True, tmpdir=tdir
    )
    out = np.empty((N, B, H), dtype=np.float32)
    for c in range(NCORES):
        out[:, c, :] = res.results[c]["out"]
    return out, res
